# revision 1
# baseline (speedup 1.0000x reference)
"""Autoformer encoder layer on 8 Trainium2 NeuronCores (Bass/Tile).

Data-parallel over batch: each of the 8 cores processes 4 of the 32 batches.
Everything runs on-device in one NEFF, including the cross-core reduction for
the global top-k delay selection (AllReduce of the batch-summed correlation
spectrum) and the data-dependent delay rolls (dynamic-offset SBUF reads).

Math notes
----------
The reference only ever uses two reductions of the full per-(head,channel)
autocorrelation:
  * mean_value[b, l] = mean over channels of irfft(rfft(q)*conj(rfft(k)))
    -> computed here as S[b,f] = sum_c Qf*conj(Kf) via DFT-as-matmul over L,
       then one tiny irfft matmul.  (The full [B,H,E,L] corr is never needed.)
  * agg = sum_i softmax(w)_i * roll(v, -d_i)  -> rolls commute with the
    output projection, so v@Wo is computed directly with folded weights
    Wvo = Wv @ Wo and rolled instead (7 dynamic-slice MACs per channel chunk).
The moving-average decomposition runs as a cumsum scan along the free axis in
channel-major layout, so the whole residual/FFN pipeline needs no on-device
transposes: x is fed pre-transposed as [B, D, L] and the output is returned
transposed, undone on the host.
"""

import os
from contextlib import ExitStack

import numpy as np

import concourse.bass as bass
import concourse.bacc as bacc
import concourse.mybir as mybir
from concourse import tile
from concourse.bass_utils import run_bass_kernel_spmd
from concourse.ordered_set import OrderedSet

F32 = mybir.dt.float32
F32R = mybir.dt.float32r
U32 = mybir.dt.uint32
AX = mybir.AxisListType
OP = mybir.AluOpType
AF = mybir.ActivationFunctionType
DVE = mybir.EngineType.DVE
SPROD_MODE = os.environ.get("KSPROD", "stt")

B, L, D, DFF = 32, 1536, 512, 2048
KMA = 25              # moving-average window
PAD = (KMA - 1) // 2  # 12
TOPK = 7              # int(1 * log(1536))
N_CORES = 8
BC = B // N_CORES     # batches per core
LF = L // 2 + 1       # 769 rfft bins
NLC = L // 128        # 12 l-chunks
NDC = D // 128        # 4 channel chunks
NFC = DFF // 128      # 16 ffn chunks
NFT = (LF + 127) // 128  # 7 f-tiles (last has 1 row)
NLB = L // 512        # 3 l-blocks of 512


def _host_consts():
    lv = np.arange(L)
    fv = np.arange(LF)
    ang = 2.0 * np.pi * np.outer(lv, fv) / L
    Cc = np.cos(ang).astype(np.float32)           # [L, LF] rfft real basis
    Cs = (-np.sin(ang)).astype(np.float32)        # [L, LF] rfft imag basis
    # irfft with the channel-mean folded in:
    # mv[l] = (1/(L*D)) sum_f beta_f (Sre[f] cos(wfl) - Sim[f] sin(wfl))
    # moving-average edge coefficients, pre-negated for fused (coef*edge)+rest
    coefL = np.tile((-(PAD - np.arange(PAD)) / KMA).astype(np.float32), (128, 1))
    coefR = np.tile((-(np.arange(PAD) + 1) / KMA).astype(np.float32), (128, 1))
    return Cc, Cs, None, coefL, coefR


def build(dbg=False):
    phases = int(os.environ.get("KPHASES", "2"))
    p1ft = int(os.environ.get("KP1FT", str(NFT)))
    p1b = int(os.environ.get("KP1B", str(BC)))
    kreps = int(os.environ.get("KREPS", "1"))
    kar = int(os.environ.get("KAR", "1"))
    nc = bacc.Bacc("TRN2", target_bir_lowering=False, debug=False, num_devices=N_CORES)

    xT = nc.dram_tensor("xT", [BC, D, L], F32R, kind="ExternalInput")
    Wq_d = nc.dram_tensor("Wq", [D, D], F32R, kind="ExternalInput")
    Wk_d = nc.dram_tensor("Wk", [D, D], F32R, kind="ExternalInput")
    Wvo_d = nc.dram_tensor("Wvo", [D, D], F32R, kind="ExternalInput")
    W1_d = nc.dram_tensor("W1", [D, DFF], F32R, kind="ExternalInput")
    W2_d = nc.dram_tensor("W2", [DFF, D], F32R, kind="ExternalInput")
    bq_d = nc.dram_tensor("bq", [1, D], F32, kind="ExternalInput")
    bk_d = nc.dram_tensor("bk", [1, D], F32, kind="ExternalInput")
    # channel-major biases prepacked host-side as [128, nchunks]
    bvo_d = nc.dram_tensor("bvo", [128, NDC], F32, kind="ExternalInput")
    b1_d = nc.dram_tensor("b1", [128, NFC], F32, kind="ExternalInput")
    b2_d = nc.dram_tensor("b2", [128, NDC], F32, kind="ExternalInput")
    Cc_d = nc.dram_tensor("Cc", [L, LF], F32R, kind="ExternalInput")
    Cs_d = nc.dram_tensor("Cs", [L, LF], F32R, kind="ExternalInput")
    coefL_d = nc.dram_tensor("coefL", [128, PAD], F32, kind="ExternalInput")
    coefR_d = nc.dram_tensor("coefR", [128, PAD], F32, kind="ExternalInput")

    resT = nc.dram_tensor("resT", [BC, D, L], F32, kind="ExternalOutput")
    if dbg:
        s_dbg = nc.dram_tensor("s_dbg", [128, 10 * NFT], F32, kind="ExternalOutput")
        mv_dbg = nc.dram_tensor("mv_dbg", [5, L], F32, kind="ExternalOutput")
        idx_dbg = nc.dram_tensor("idx_dbg", [1, 8], U32, kind="ExternalOutput")
        w_dbg = nc.dram_tensor("w_dbg", [BC, TOPK], F32, kind="ExternalOutput")

    with tile.TileContext(nc) as tc, ExitStack() as stack:
        pp = stack.enter_context(tc.tile_pool(name="persist", bufs=1))
        dram = stack.enter_context(tc.tile_pool(name="dram", bufs=1, space="DRAM"))

        # ---- persistent biases --------------------------------------------
        bq_bc = pp.tile([128, D], F32, tag="bqbc")
        bk_bc = pp.tile([128, D], F32, tag="bkbc")
        brow = pp.tile([1, D], F32, tag="brow")
        nc.sync.dma_start(out=brow[:, :], in_=bq_d[:, :])
        nc.gpsimd.partition_broadcast(bq_bc[:, :], brow[0:1, :])
        brow2 = pp.tile([1, D], F32, tag="brow2")
        nc.sync.dma_start(out=brow2[:, :], in_=bk_d[:, :])
        nc.gpsimd.partition_broadcast(bk_bc[:, :], brow2[0:1, :])

        bvoT = pp.tile([128, NDC], F32, tag="bvoT")
        b1T = pp.tile([128, NFC], F32, tag="b1T")
        b2T = pp.tile([128, NDC], F32, tag="b2T")
        nc.sync.dma_start(out=bvoT[:, :], in_=bvo_d[:, :])
        nc.sync.dma_start(out=b1T[:, :], in_=b1_d[:, :])
        nc.sync.dma_start(out=b2T[:, :], in_=b2_d[:, :])

        coefL_sb = pp.tile([128, PAD], F32, tag="coefL")
        coefR_sb = pp.tile([128, PAD], F32, tag="coefR")
        nc.sync.dma_start(out=coefL_sb[:, :], in_=coefL_d[:, :])
        nc.sync.dma_start(out=coefR_sb[:, :], in_=coefR_d[:, :])

        # identity matrix for PE transposes + irfft row scales
        idint = pp.tile([128, 128], mybir.dt.int32, tag="idint")
        nc.gpsimd.iota(idint[:, :], pattern=[[1, 128]], base=0, channel_multiplier=-1)
        ident = pp.tile([128, 128], F32, tag="ident")
        nc.vector.tensor_scalar(ident[:, :], idint[:, :], 0, None, op0=OP.is_equal)
        betac = pp.tile([128, 1], F32, tag="betac")
        nc.vector.memset(betac[:, :], 2.0 / (L * D))
        betac0 = pp.tile([128, 1], F32, tag="betac0")
        nc.vector.memset(betac0[:, :], 2.0 / (L * D))
        nc.vector.memset(betac0[0:1, :], 1.0 / (L * D))

        # spectrum accumulator: 7 f-tiles x (4 batches + batchsum) columns
        s_re = pp.tile([128, 5 * NFT], F32, tag="s_re")
        s_im = pp.tile([128, 5 * NFT], F32, tag="s_im")
        nc.vector.memset(s_re[:, :], 0.0)
        nc.vector.memset(s_im[:, :], 0.0)

        mv_sb = pp.tile([5, L], F32, tag="mv")  # rows 0-3: batches, 4: batchsum
        idx_sb = pp.tile([1, 8], U32, tag="idx")
        wbc = pp.tile([128, BC * TOPK], F32, tag="wbc")
        wv = pp.tile([BC, TOPK], F32, tag="wv")

        for _rep in range(kreps):
            # ================= PHASE 1: q/k projections + DFT spectrum =========
            with ExitStack() as p1stack:
                p1c = p1stack.enter_context(tc.tile_pool(name="p1c", bufs=1))
                wq_sb, wk_sb = [], []
                for c in range(NDC):
                    t = p1c.tile([128, D], F32R, tag=f"wq{c}")
                    nc.sync.dma_start(out=t[:, :], in_=Wq_d[128 * c : 128 * (c + 1), :])
                    wq_sb.append(t)
                    t = p1c.tile([128, D], F32R, tag=f"wk{c}")
                    nc.sync.dma_start(out=t[:, :], in_=Wk_d[128 * c : 128 * (c + 1), :])
                    wk_sb.append(t)
                cc_sb, cs_sb = [], []
                for lc in range(NLC):
                    t = p1c.tile([128, LF], F32R, tag=f"cc{lc}")
                    nc.sync.dma_start(out=t[:, :], in_=Cc_d[128 * lc : 128 * (lc + 1), :])
                    cc_sb.append(t)
                    t = p1c.tile([128, LF], F32R, tag=f"cs{lc}")
                    nc.sync.dma_start(out=t[:, :], in_=Cs_d[128 * lc : 128 * (lc + 1), :])
                    cs_sb.append(t)

                pbatch = ExitStack()
                p1x = pbatch.enter_context(tc.tile_pool(name="p1x", bufs=1))
                p1q = pbatch.enter_context(tc.tile_pool(name="p1q", bufs=1))
                p1e = pbatch.enter_context(tc.tile_pool(name="p1e", bufs=1))
                p1s = pbatch.enter_context(tc.tile_pool(name="p1s", bufs=2))
                ps1 = pbatch.enter_context(tc.tile_pool(name="psum1", bufs=2, space="PSUM"))
                ps1d = pbatch.enter_context(tc.tile_pool(name="psum1d", bufs=1, space="PSUM"))

                for b in range(p1b):
                    xt = [p1x.tile([128, L], F32R, tag=f"xt{c}", name=f"xt{c}") for c in range(NDC)]
                    for c in range(NDC):
                        nc.sync.dma_start(
                            out=xt[c][:, :], in_=xT[b, 128 * c : 128 * (c + 1), :])

                    q_sb, k_sb = [], []
                    for lt in range(NLC):
                        pq = ps1.tile([128, D], F32, tag="pq")
                        pk = ps1.tile([128, D], F32, tag="pk")
                        for c in range(NDC):
                            nc.tensor.matmul(
                                pq[:, :], xt[c][:, 128 * lt : 128 * (lt + 1)],
                                wq_sb[c][:, :], start=(c == 0), stop=(c == NDC - 1))
                        for c in range(NDC):
                            nc.tensor.matmul(
                                pk[:, :], xt[c][:, 128 * lt : 128 * (lt + 1)],
                                wk_sb[c][:, :], start=(c == 0), stop=(c == NDC - 1))
                        qt = p1q.tile([128, D], F32R, tag=f"q{lt}")
                        kt = p1q.tile([128, D], F32R, tag=f"k{lt}")
                        nc.vector.tensor_add(qt[:, :], pq[:, :], bq_bc[:, :])
                        nc.vector.tensor_add(kt[:, :], pk[:, :], bk_bc[:, :])
                        q_sb.append(qt)
                        k_sb.append(kt)

                    for ft in range(p1ft):
                        m = min(128, LF - 128 * ft)
                        fsl = slice(128 * ft, 128 * ft + m)
                        pqr = ps1d.tile([128, D], F32, tag="pqr")
                        pqi = ps1d.tile([128, D], F32, tag="pqi")
                        pkr = ps1d.tile([128, D], F32, tag="pkr")
                        pki = ps1d.tile([128, D], F32, tag="pki")
                        for lc in range(NLC):
                            st_ = lc == 0
                            sp_ = lc == NLC - 1
                            nc.tensor.matmul(pqr[:m, :], cc_sb[lc][:, fsl],
                                             q_sb[lc][:, :], start=st_, stop=sp_)
                            nc.tensor.matmul(pqi[:m, :], cs_sb[lc][:, fsl],
                                             q_sb[lc][:, :], start=st_, stop=sp_)
                            nc.tensor.matmul(pkr[:m, :], cc_sb[lc][:, fsl],
                                             k_sb[lc][:, :], start=st_, stop=sp_)
                            nc.tensor.matmul(pki[:m, :], cs_sb[lc][:, fsl],
                                             k_sb[lc][:, :], start=st_, stop=sp_)
                        qr = p1e.tile([128, D], F32, tag="qr")
                        qi = p1e.tile([128, D], F32, tag="qi")
                        kr = p1e.tile([128, D], F32, tag="kr")
                        ki = p1e.tile([128, D], F32, tag="ki")
                        nc.scalar.copy(qr[:m, :], pqr[:m, :])
                        nc.scalar.copy(qi[:m, :], pqi[:m, :])
                        nc.scalar.copy(kr[:m, :], pkr[:m, :])
                        nc.scalar.copy(ki[:m, :], pki[:m, :])
                        scr = p1s.tile([128, D], F32, tag="scr")
                        acc = p1s.tile([128, 4], F32, tag="acc")
                        col = 5 * ft + b
                        prods = ((qr, kr, 0), (qi, ki, 1), (qi, kr, 2), (qr, ki, 3))
                        if SPROD_MODE == "stt":
                            for pa, pb, j in prods:
                                nc.vector.scalar_tensor_tensor(
                                    out=scr[:m, :], in0=pa[:m, :], scalar=1.0,
                                    in1=pb[:m, :], op0=OP.mult, op1=OP.mult,
                                    accum_out=acc[:m, j : j + 1])
                        else:
                            for pa, pb, j in prods:
                                nc.vector.tensor_mul(scr[:m, :], pa[:m, :], pb[:m, :])
                                nc.vector.reduce_sum(
                                    acc[:m, j : j + 1], scr[:m, :], axis=AX.X)
                        nc.vector.tensor_add(
                            s_re[:m, col : col + 1], acc[:m, 0:1], acc[:m, 1:2])
                        nc.vector.tensor_sub(
                            s_im[:m, col : col + 1], acc[:m, 2:3], acc[:m, 3:4])

                pbatch.close()
                # ================= PHASE 1.5: irfft, allreduce, topk, weights ======
                if dbg:
                    nc.sync.dma_start(out=s_dbg[:, 0 : 5 * NFT], in_=s_re[:, :])
                    nc.sync.dma_start(out=s_dbg[:, 5 * NFT : 10 * NFT], in_=s_im[:, :])
                if phases < 0:
                    nc.vector.memset(mv_sb[:, :], 0.0)
                    nc.vector.memset(idx_sb[:, :], 0)
                    nc.vector.memset(wbc[:, :], 0.0)
                    nc.vector.memset(wv[:, :], 0.0)
                nftr = range(NFT) if phases >= 0 else range(0)
                for ft in nftr:
                    nc.vector.reduce_sum(
                        s_re[:, 5 * ft + 4 : 5 * ft + 5], s_re[:, 5 * ft : 5 * ft + 4],
                        axis=AX.X)
                    nc.vector.reduce_sum(
                        s_im[:, 5 * ft + 4 : 5 * ft + 5], s_im[:, 5 * ft : 5 * ft + 4],
                        axis=AX.X)

                with (
                    tc.tile_pool(name="irf", bufs=3) as irf,
                    tc.tile_pool(name="psum15", bufs=2, space="PSUM") as ps15,
                    tc.tile_pool(name="psum15t", bufs=2, space="PSUM") as ps15t,
                ):
                    # Cc/Cs are still resident; derive the irfft operand
                    # (beta_f/(L*D) * [cos|-sin]^T) by transposing their tiles.
                    for nb in range(NLB) if phases >= 0 else range(0):
                        pmv = ps15.tile([5, 512], F32, tag="pmv")
                        first = True
                        for half, stile, csb in ((0, s_re, cc_sb), (1, s_im, cs_sb)):
                            for ft in range(NFT):
                                m = min(128, LF - 128 * ft)
                                fsl = slice(128 * ft, 128 * ft + m)
                                pt = ps15t.tile([128, 512], F32, tag="pt")
                                for j in range(4):
                                    lc = 4 * nb + j
                                    nc.tensor.transpose(
                                        pt[:m, 128 * j : 128 * (j + 1)],
                                        csb[lc][:, fsl].bitcast(F32),
                                        ident[:, :])
                                mt = irf.tile([128, 512], F32, tag="minv")
                                if m < 128:
                                    nc.vector.memset(mt[:, :], 0.0)
                                bcol = betac0 if (ft == 0 or ft == NFT - 1) else betac
                                nc.scalar.activation(
                                    mt[:m, :], pt[:m, :], AF.Copy, scale=bcol[:m, 0:1])
                                nc.tensor.matmul(
                                    pmv[:, :], stile[:, 5 * ft : 5 * ft + 5], mt[:, :],
                                    start=first, stop=(half == 1 and ft == NFT - 1))
                                first = False
                        nc.vector.tensor_copy(mv_sb[:, 512 * nb : 512 * (nb + 1)], pmv[:, :])

            # AllReduce the batch-summed spectrum row -> global over all 32 batches
            do_ar = phases >= 0 and kar != 0
            cc_in = dram.tile([1, L], F32)
            cc_out = dram.tile([1, L], F32)
            mvg = pp.tile([1, L], F32, tag="mvg")
            max8 = pp.tile([1, 8], F32, tag="max8")
            if phases >= 0 and not do_ar:
                nc.vector.memset(idx_sb[:, :], 0)
            if do_ar:
                nc.sync.dma_start(out=cc_in[:, :], in_=mv_sb[4:5, :])
                nc.gpsimd.collective_compute(
                    "AllReduce",
                    OP.add,
                    replica_groups=[list(range(N_CORES))],
                    ins=[cc_in[:, :].opt()],
                    outs=[cc_out[:, :].opt()],
                )
                nc.sync.dma_start(out=mvg[:, :], in_=cc_out[:, :])
                nc.vector.max(out=max8[:, :], in_=mvg[:, :])
                nc.vector.max_index(out=idx_sb[:, :], in_max=max8[:, :], in_values=mvg[:, :])

            ntk = range(TOPK) if phases >= 1 else range(0)
            dvals = [
                nc.values_load(
                    idx_sb[0:1, i : i + 1],
                    engines=OrderedSet([DVE]),
                    min_val=0,
                    max_val=L - 1,
                    skip_runtime_bounds_check=True,
                )
                for i in ntk
            ]

            # per-batch weights at the selected delays + softmax, then broadcast
            if 0 <= phases < 1:
                nc.vector.memset(wv[:, :], 0.0)
            for i in ntk:
                nc.vector.tensor_copy(wv[:, i : i + 1], mv_sb[0:BC, bass.ds(dvals[i], 1)])
            wred = pp.tile([BC, 2], F32, tag="wred")
            if phases < 1:
                nc.vector.memset(wbc[:, :], 0.0)
            if phases >= 1:
                nc.vector.reduce_max(wred[:, 0:1], wv[:, :], axis=AX.X)
                wexp = pp.tile([BC, TOPK], F32, tag="wexp")
                nc.vector.tensor_scalar(
                    wexp[:, :], wv[:, :], wred[:, 0:1], None, op0=OP.subtract)
                nc.scalar.activation(wexp[:, :], wexp[:, :], AF.Exp)
                nc.vector.reduce_sum(wred[:, 1:2], wexp[:, :], axis=AX.X)
                nc.vector.reciprocal(wred[:, 1:2], wred[:, 1:2])
                nc.vector.tensor_scalar(
                    wexp[:, :], wexp[:, :], wred[:, 1:2], None, op0=OP.mult)
                w_dram = dram.tile([BC, TOPK], F32)
                nc.sync.dma_start(out=w_dram[:, :], in_=wexp[:, :])
                wflat = pp.tile([1, BC * TOPK], F32, tag="wflat")
                nc.sync.dma_start(out=wflat[:, :], in_=w_dram[:, :])
                nc.gpsimd.partition_broadcast(wbc[:, :], wflat[0:1, :])

            if dbg:
                nc.sync.dma_start(out=mv_dbg[:, :], in_=mv_sb[:, :])
                nc.sync.dma_start(out=idx_dbg[:, :], in_=idx_sb[:, :])
                nc.sync.dma_start(out=w_dbg[:, :], in_=wexp[:, :] if phases >= 1 else wv[:, :])

            # ================= PHASE 2: rolls, decomp, FFN, decomp =============
            def ma_seasonal(pool, dst, src):
                """dst = src - moving_avg(src) along the free axis (edge-replicated).

                src must be F32-readable; dst may be F32 or F32R."""
                cs1 = pool.tile([128, L + 1], F32, tag="cs1", bufs=1)
                nc.vector.memset(cs1[:, 0:1], 0.0)
                nc.vector.tensor_tensor_scan(
                    cs1[:, 1 : L + 1], src[:, :], src[:, :], 0.0,
                    op0=OP.add, op1=OP.bypass)
                dif = pool.tile([128, L - 2 * PAD], F32, tag="dif", bufs=1)
                nc.vector.tensor_sub(
                    dif[:, :], cs1[:, 2 * PAD + 1 : L + 1], cs1[:, 0 : L - 2 * PAD])
                nc.vector.scalar_tensor_tensor(
                    out=dst[:, PAD : L - PAD], in0=dif[:, :], scalar=-1.0 / KMA,
                    in1=src[:, PAD : L - PAD], op0=OP.mult, op1=OP.add)
                # left edge: s[l] = x[l] - cs1[l+PAD+1]/K - (PAD-l)/K * x[0]
                nc.vector.scalar_tensor_tensor(
                    out=dst[:, 0:PAD], in0=cs1[:, PAD + 1 : 2 * PAD + 1],
                    scalar=-1.0 / KMA, in1=src[:, 0:PAD], op0=OP.mult, op1=OP.add)
                nc.vector.scalar_tensor_tensor(
                    out=dst[:, 0:PAD], in0=coefL_sb[:, :], scalar=src[:, 0:1],
                    in1=dst[:, 0:PAD], op0=OP.mult, op1=OP.add)
                # right edge: s[l] = x[l] - (stot - cs1[l-PAD])/K - (l-L+PAD+1)/K * x[L-1]
                e2 = pool.tile([128, PAD], F32, tag="e2", bufs=1)
                nc.vector.tensor_scalar(
                    e2[:, :], cs1[:, L - 2 * PAD : L - PAD], cs1[:, L : L + 1],
                    1.0 / KMA, op0=OP.subtract, op1=OP.mult)
                nc.vector.tensor_add(
                    dst[:, L - PAD : L], e2[:, :], src[:, L - PAD : L])
                nc.vector.scalar_tensor_tensor(
                    out=dst[:, L - PAD : L], in0=coefR_sb[:, :],
                    scalar=src[:, L - 1 : L], in1=dst[:, L - PAD : L],
                    op0=OP.mult, op1=OP.add)

            with ExitStack() as p2stack:
                nch = range(NDC) if phases >= 2 else range(0)
                nfh = range(NFC) if phases >= 2 else range(0)
                p2w = p2stack.enter_context(tc.tile_pool(name="p2w", bufs=1))
                w1_sb = []
                for c in nch:
                    t = p2w.tile([128, DFF], F32R, tag=f"w1_{c}")
                    nc.sync.dma_start(out=t[:, :], in_=W1_d[128 * c : 128 * (c + 1), :])
                    w1_sb.append(t)
                w2_sb = []
                for c in nfh:
                    t = p2w.tile([128, D], F32R, tag=f"w2_{c}")
                    nc.sync.dma_start(out=t[:, :], in_=W2_d[128 * c : 128 * (c + 1), :])
                    w2_sb.append(t)
                wvo_sb = []
                for c in nch:
                    t = p2w.tile([128, D], F32R, tag=f"wvo{c}")
                    nc.sync.dma_start(out=t[:, :], in_=Wvo_d[128 * c : 128 * (c + 1), :])
                    wvo_sb.append(t)

                p2x = p2stack.enter_context(tc.tile_pool(name="p2x", bufs=1))
                p2 = p2stack.enter_context(tc.tile_pool(name="p2", bufs=1))
                ps2 = p2stack.enter_context(tc.tile_pool(name="psum2", bufs=2, space="PSUM"))
                ps2y = p2stack.enter_context(tc.tile_pool(name="psum2y", bufs=1, space="PSUM"))

                for b in range(BC) if phases >= 2 else range(0):
                    xt = [p2x.tile([128, L], F32R, tag=f"x2t{c}", name=f"x2t{c}") for c in range(NDC)]
                    for c in range(NDC):
                        nc.sync.dma_start(
                            out=xt[c][:, :], in_=xT[b, 128 * c : 128 * (c + 1), :])

                    # vo' = x @ (Wv Wo); x2 = x + bvo + sum_i w_i roll(vo', d_i)
                    x2 = [p2.tile([128, L], F32, tag=f"x2_{c}", name=f"x2_{c}") for c in range(NDC)]
                    for c in range(NDC):
                        vo2 = p2.tile([128, 2 * L], F32, tag="vo2", bufs=1)
                        for nb in range(NLB):
                            pv = ps2.tile([128, 512], F32, tag="pv")
                            for cx in range(NDC):
                                nc.tensor.matmul(
                                    pv[:, :],
                                    wvo_sb[cx][:, 128 * c : 128 * (c + 1)],
                                    xt[cx][:, 512 * nb : 512 * (nb + 1)],
                                    start=(cx == 0),
                                    stop=(cx == NDC - 1),
                                )
                            nc.scalar.copy(vo2[:, 512 * nb : 512 * (nb + 1)], pv[:, :])
                            nc.scalar.copy(
                                vo2[:, L + 512 * nb : L + 512 * (nb + 1)], pv[:, :])
                        nc.scalar.activation(
                            x2[c][:, :], xt[c][:, :], AF.Identity,
                            bias=bvoT[:, c : c + 1])
                        for i in range(TOPK):
                            nc.vector.scalar_tensor_tensor(
                                out=x2[c][:, :],
                                in0=vo2[:, bass.ds(dvals[i], L)],
                                scalar=wbc[:, TOPK * b + i : TOPK * b + i + 1],
                                in1=x2[c][:, :],
                                op0=OP.mult,
                                op1=OP.add,
                            )

                    # first decomposition -> seasonal part s (f32r: feeds FFN)
                    st = [p2.tile([128, L], F32R, tag=f"st{c}", name=f"st{c}") for c in range(NDC)]
                    for c in range(NDC):
                        ma_seasonal(p2, st[c], x2[c])

                    # FFN + residual: z = s + relu(s W1 + b1) W2 + b2
                    z = x2  # reuse buffers
                    for nb in range(NLB):
                        lsl = slice(512 * nb, 512 * (nb + 1))
                        py = [ps2y.tile([128, 512], F32, tag=f"py{c}", name=f"py{c}") for c in range(NDC)]
                        for fc in range(NFC):
                            ph = ps2.tile([128, 512], F32, tag="ph")
                            for c in range(NDC):
                                nc.tensor.matmul(
                                    ph[:, :],
                                    w1_sb[c][:, 128 * fc : 128 * (fc + 1)],
                                    st[c][:, lsl],
                                    start=(c == 0),
                                    stop=(c == NDC - 1),
                                )
                            ht = p2.tile([128, 512], F32R, tag="ht", bufs=3)
                            nc.scalar.activation(
                                ht[:, :], ph[:, :], AF.Relu, bias=b1T[:, fc : fc + 1])
                            for c in range(NDC):
                                nc.tensor.matmul(
                                    py[c][:, :],
                                    w2_sb[fc][:, 128 * c : 128 * (c + 1)],
                                    ht[:, :],
                                    start=(fc == 0),
                                    stop=(fc == NFC - 1),
                                )
                        for c in range(NDC):
                            nc.vector.scalar_tensor_tensor(
                                out=z[c][:, lsl], in0=py[c][:, :],
                                scalar=b2T[:, c : c + 1], in1=st[c][:, lsl],
                                op0=OP.add, op1=OP.add)

                    # second decomposition -> output
                    for c in range(NDC):
                        rt = p2.tile([128, L], F32, tag="rt", bufs=1)
                        ma_seasonal(p2, rt, z[c])
                        nc.sync.dma_start(
                            out=resT[b, 128 * c : 128 * (c + 1), :], in_=rt[:, :])

    nc.compile()
    return nc


_CACHE = {}


def _get_nc(dbg=False):
    if dbg not in _CACHE:
        _CACHE[dbg] = build(dbg=dbg)
    return _CACHE[dbg]


def make_in_maps(x, Wq, bq, Wk, bk, Wv, bv, Wo, bo, W1, b1, W2, b2):
    Cc_np, Cs_np, Minv_np, coefL_np, coefR_np = _host_consts()
    x = np.asarray(x, np.float32)
    Wvo = (np.asarray(Wv, np.float64) @ np.asarray(Wo, np.float64)).astype(np.float32)
    bvo = (np.asarray(bv, np.float64) @ np.asarray(Wo, np.float64)
           + np.asarray(bo, np.float64)).astype(np.float32)
    shared = {
        "Wq": np.ascontiguousarray(Wq, np.float32),
        "Wk": np.ascontiguousarray(Wk, np.float32),
        "Wvo": Wvo,
        "W1": np.ascontiguousarray(W1, np.float32),
        "W2": np.ascontiguousarray(W2, np.float32),
        "bq": np.asarray(bq, np.float32).reshape(1, D),
        "bk": np.asarray(bk, np.float32).reshape(1, D),
        "bvo": np.ascontiguousarray(bvo.reshape(NDC, 128).T),
        "b1": np.ascontiguousarray(np.asarray(b1, np.float32).reshape(NFC, 128).T),
        "b2": np.ascontiguousarray(np.asarray(b2, np.float32).reshape(NDC, 128).T),
        "Cc": Cc_np,
        "Cs": Cs_np,
        "coefL": coefL_np,
        "coefR": coefR_np,
    }
    in_maps = []
    for c in range(N_CORES):
        xs = x[BC * c : BC * (c + 1)]
        in_maps.append({**shared, "xT": np.ascontiguousarray(xs.transpose(0, 2, 1))})
    return in_maps


def run(inputs, dbg=False, trace=False):
    nc = _get_nc(dbg=dbg)
    in_maps = make_in_maps(**inputs)
    res = run_bass_kernel_spmd(
        nc, in_maps, core_ids=list(range(N_CORES)), trace=trace)
    out = np.empty((B, L, D), np.float32)
    for c in range(N_CORES):
        rt = res.results[c]["resT"]  # [BC, D, L]
        out[BC * c : BC * (c + 1)] = rt.transpose(0, 2, 1)
    return out, res


def kernel(**inputs):
    out, _ = run(inputs)
    return out


_NULL_CACHE = {}


def _get_null_nc():
    if "nc" not in _NULL_CACHE:
        nc = bacc.Bacc("TRN2", target_bir_lowering=False, debug=False,
                       num_devices=N_CORES)
        ins = {
            "xT": [BC, D, L], "Wq": [D, D], "Wk": [D, D], "Wvo": [D, D],
            "W1": [D, DFF], "W2": [DFF, D], "bq": [1, D], "bk": [1, D],
            "bvo": [128, NDC], "b1": [128, NFC], "b2": [128, NDC],
            "Cc": [L, LF], "Cs": [L, LF],
            "coefL": [128, PAD], "coefR": [128, PAD],
        }
        for name, shape in ins.items():
            nc.dram_tensor(name, shape, F32, kind="ExternalInput")
        resT = nc.dram_tensor("resT", [BC, D, L], F32, kind="ExternalOutput")
        with tile.TileContext(nc) as tc:
            with tc.tile_pool(name="sb", bufs=1) as sb:
                t = sb.tile([128, 4], F32, name="t")
                nc.vector.memset(t[:, :], 0.0)
                nc.sync.dma_start(out=resT[0, 0:128, 0:4], in_=t[:, :])
        nc.compile()
        _NULL_CACHE["nc"] = nc
    return _NULL_CACHE["nc"]


def time_null(inputs, reps=3):
    import time as _time
    nc = _get_null_nc()
    in_maps = make_in_maps(**inputs)
    run_bass_kernel_spmd(nc, in_maps, core_ids=list(range(N_CORES)))
    best = float("inf")
    for _ in range(reps):
        t0 = _time.time()
        run_bass_kernel_spmd(nc, in_maps, core_ids=list(range(N_CORES)))
        best = min(best, _time.time() - t0)
    return best


def time_main(inputs, reps=3, dbg=False):
    import time as _time
    nc = _get_nc(dbg=dbg)
    in_maps = make_in_maps(**inputs)
    run_bass_kernel_spmd(nc, in_maps, core_ids=list(range(N_CORES)))
    best = float("inf")
    for _ in range(reps):
        t0 = _time.time()
        run_bass_kernel_spmd(nc, in_maps, core_ids=list(range(N_CORES)))
        best = min(best, _time.time() - t0)
    return best



# revision 4
# speedup vs baseline: 247.3868x; 247.3868x over previous
"""Autoformer encoder layer on 8 Trainium2 NeuronCores (Bass/Tile).

Data-parallel over batch: each of the 8 cores processes 4 of the 32 batches.
Everything runs on-device in one NEFF, including the cross-core reduction for
the global top-k delay selection (AllReduce of the batch-summed correlation
spectrum) and the data-dependent delay rolls (dynamic-offset SBUF reads).

Math notes
----------
The reference only ever uses two reductions of the full per-(head,channel)
autocorrelation:
  * mean_value[b, l] = mean over channels of irfft(rfft(q)*conj(rfft(k)))
    -> computed here as S[b,f] = sum_c Qf*conj(Kf) via DFT-as-matmul over L,
       then one tiny irfft matmul.  (The full [B,H,E,L] corr is never needed.)
  * agg = sum_i softmax(w)_i * roll(v, -d_i)  -> rolls commute with the
    output projection, so v@Wo is computed directly with folded weights
    Wvo = Wv @ Wo and rolled instead (7 dynamic-slice MACs per channel chunk).
The moving-average decomposition runs as a cumsum scan along the free axis in
channel-major layout, so the whole residual/FFN pipeline needs no on-device
transposes: x is fed pre-transposed as [B, D, L] and the output is returned
transposed, undone on the host.
"""

import os
from contextlib import ExitStack

import numpy as np

import concourse.bass as bass
import concourse.bacc as bacc
import concourse.mybir as mybir
from concourse import tile
from concourse.bass_utils import run_bass_kernel_spmd
from concourse.ordered_set import OrderedSet

F32 = mybir.dt.float32
F32R = mybir.dt.float32r
U32 = mybir.dt.uint32
AX = mybir.AxisListType
OP = mybir.AluOpType
AF = mybir.ActivationFunctionType
DVE = mybir.EngineType.DVE
SPROD_MODE = os.environ.get("KSPROD", "stt")

B, L, D, DFF = 32, 1536, 512, 2048
KMA = 25              # moving-average window
PAD = (KMA - 1) // 2  # 12
TOPK = 7              # int(1 * log(1536))
N_CORES = 8
BC = B // N_CORES     # batches per core
LF = L // 2 + 1       # 769 rfft bins
NLC = L // 128        # 12 l-chunks
NDC = D // 128        # 4 channel chunks
NFC = DFF // 128      # 16 ffn chunks
NFT = (LF + 127) // 128  # 7 f-tiles (last has 1 row)
NLB = L // 512        # 3 l-blocks of 512


def _host_consts():
    lv = np.arange(L)
    fv = np.arange(LF)
    ang = 2.0 * np.pi * np.outer(lv, fv) / L
    Cc = np.cos(ang).astype(np.float32)           # [L, LF] rfft real basis
    Cs = (-np.sin(ang)).astype(np.float32)        # [L, LF] rfft imag basis
    # irfft with the channel-mean folded in:
    # mv[l] = (1/(L*D)) sum_f beta_f (Sre[f] cos(wfl) - Sim[f] sin(wfl))
    # moving-average edge coefficients, pre-negated for fused (coef*edge)+rest
    coefL = np.tile((-(PAD - np.arange(PAD)) / KMA).astype(np.float32), (128, 1))
    coefR = np.tile((-(np.arange(PAD) + 1) / KMA).astype(np.float32), (128, 1))
    return Cc, Cs, None, coefL, coefR


def build(dbg=False):
    phases = int(os.environ.get("KPHASES", "2"))
    p1ft = int(os.environ.get("KP1FT", str(NFT)))
    p1b = int(os.environ.get("KP1B", str(BC)))
    kreps = int(os.environ.get("KREPS", "1"))
    kar = int(os.environ.get("KAR", "1"))
    nc = bacc.Bacc("TRN2", target_bir_lowering=False, debug=False, num_devices=N_CORES)

    xT = nc.dram_tensor("xT", [BC, D, L], F32R, kind="ExternalInput")
    Wq_d = nc.dram_tensor("Wq", [D, D], F32R, kind="ExternalInput")
    Wk_d = nc.dram_tensor("Wk", [D, D], F32R, kind="ExternalInput")
    Wvo_d = nc.dram_tensor("Wvo", [D, D], F32R, kind="ExternalInput")
    W1_d = nc.dram_tensor("W1", [D, DFF], F32R, kind="ExternalInput")
    W2_d = nc.dram_tensor("W2", [DFF, D], F32R, kind="ExternalInput")
    bq_d = nc.dram_tensor("bq", [1, D], F32, kind="ExternalInput")
    bk_d = nc.dram_tensor("bk", [1, D], F32, kind="ExternalInput")
    # channel-major biases prepacked host-side as [128, nchunks]
    bvo_d = nc.dram_tensor("bvo", [128, NDC], F32, kind="ExternalInput")
    b1_d = nc.dram_tensor("b1", [128, NFC], F32, kind="ExternalInput")
    b2_d = nc.dram_tensor("b2", [128, NDC], F32, kind="ExternalInput")
    # shape-derived constants: baked into the NEFF, no per-call transfer
    Cc_np, Cs_np, _, coefL_np, coefR_np = _host_consts()
    Cc_d = nc.inline_tensor(Cc_np, name="Cc")
    Cs_d = nc.inline_tensor(Cs_np, name="Cs")
    coefL_d = nc.inline_tensor(coefL_np, name="coefL")
    coefR_d = nc.inline_tensor(coefR_np, name="coefR")

    resT = nc.dram_tensor("resT", [BC, D, L], F32, kind="ExternalOutput")
    if dbg:
        s_dbg = nc.dram_tensor("s_dbg", [128, 10 * NFT], F32, kind="ExternalOutput")
        mv_dbg = nc.dram_tensor("mv_dbg", [5, L], F32, kind="ExternalOutput")
        idx_dbg = nc.dram_tensor("idx_dbg", [1, 8], U32, kind="ExternalOutput")
        w_dbg = nc.dram_tensor("w_dbg", [BC, TOPK], F32, kind="ExternalOutput")

    with tile.TileContext(nc) as tc, ExitStack() as stack:
        pp = stack.enter_context(tc.tile_pool(name="persist", bufs=1))
        dram = stack.enter_context(tc.tile_pool(name="dram", bufs=1, space="DRAM"))

        # ---- persistent biases --------------------------------------------
        bq_bc = pp.tile([128, D], F32, tag="bqbc")
        bk_bc = pp.tile([128, D], F32, tag="bkbc")
        brow = pp.tile([1, D], F32, tag="brow")
        nc.sync.dma_start(out=brow[:, :], in_=bq_d[:, :])
        nc.gpsimd.partition_broadcast(bq_bc[:, :], brow[0:1, :])
        brow2 = pp.tile([1, D], F32, tag="brow2")
        nc.sync.dma_start(out=brow2[:, :], in_=bk_d[:, :])
        nc.gpsimd.partition_broadcast(bk_bc[:, :], brow2[0:1, :])

        bvoT = pp.tile([128, NDC], F32, tag="bvoT")
        b1T = pp.tile([128, NFC], F32, tag="b1T")
        b2T = pp.tile([128, NDC], F32, tag="b2T")
        nc.sync.dma_start(out=bvoT[:, :], in_=bvo_d[:, :])
        nc.sync.dma_start(out=b1T[:, :], in_=b1_d[:, :])
        nc.sync.dma_start(out=b2T[:, :], in_=b2_d[:, :])

        coefL_sb = pp.tile([128, PAD], F32, tag="coefL")
        coefR_sb = pp.tile([128, PAD], F32, tag="coefR")
        nc.sync.dma_start(out=coefL_sb[:, :], in_=coefL_d[:, :])
        nc.sync.dma_start(out=coefR_sb[:, :], in_=coefR_d[:, :])

        # identity matrix for PE transposes + irfft row scales
        idint = pp.tile([128, 128], mybir.dt.int32, tag="idint")
        nc.gpsimd.iota(idint[:, :], pattern=[[1, 128]], base=0, channel_multiplier=-1)
        ident = pp.tile([128, 128], F32, tag="ident")
        nc.vector.tensor_scalar(ident[:, :], idint[:, :], 0, None, op0=OP.is_equal)
        betac = pp.tile([128, 1], F32, tag="betac")
        nc.vector.memset(betac[:, :], 2.0 / (L * D))
        betac0 = pp.tile([128, 1], F32, tag="betac0")
        nc.vector.memset(betac0[:, :], 2.0 / (L * D))
        nc.vector.memset(betac0[0:1, :], 1.0 / (L * D))

        # spectrum accumulator: 7 f-tiles x (4 batches + batchsum) columns
        s_re = pp.tile([128, 5 * NFT], F32, tag="s_re")
        s_im = pp.tile([128, 5 * NFT], F32, tag="s_im")
        nc.vector.memset(s_re[:, :], 0.0)
        nc.vector.memset(s_im[:, :], 0.0)

        mv_sb = pp.tile([5, L], F32, tag="mv")  # rows 0-3: batches, 4: batchsum
        idx_sb = pp.tile([1, 8], U32, tag="idx")
        wbc = pp.tile([128, BC * TOPK], F32, tag="wbc")
        wv = pp.tile([BC, TOPK], F32, tag="wv")

        for _rep in range(kreps):
            # ================= PHASE 1: q/k projections + DFT spectrum =========
            with ExitStack() as p1stack:
                p1c = p1stack.enter_context(tc.tile_pool(name="p1c", bufs=1))
                wq_sb, wk_sb = [], []
                for c in range(NDC):
                    t = p1c.tile([128, D], F32R, tag=f"wq{c}")
                    nc.sync.dma_start(out=t[:, :], in_=Wq_d[128 * c : 128 * (c + 1), :])
                    wq_sb.append(t)
                    t = p1c.tile([128, D], F32R, tag=f"wk{c}")
                    nc.sync.dma_start(out=t[:, :], in_=Wk_d[128 * c : 128 * (c + 1), :])
                    wk_sb.append(t)
                cc_sb, cs_sb = [], []
                for lc in range(NLC):
                    t = p1c.tile([128, LF], F32R, tag=f"cc{lc}")
                    nc.sync.dma_start(
                        out=t[:, :],
                        in_=Cc_d[128 * lc : 128 * (lc + 1), :].bitcast(F32R))
                    cc_sb.append(t)
                    t = p1c.tile([128, LF], F32R, tag=f"cs{lc}")
                    nc.sync.dma_start(
                        out=t[:, :],
                        in_=Cs_d[128 * lc : 128 * (lc + 1), :].bitcast(F32R))
                    cs_sb.append(t)

                pbatch = ExitStack()
                p1x = pbatch.enter_context(tc.tile_pool(name="p1x", bufs=1))
                p1q = pbatch.enter_context(tc.tile_pool(name="p1q", bufs=1))
                p1e = pbatch.enter_context(tc.tile_pool(name="p1e", bufs=1))
                p1s = pbatch.enter_context(tc.tile_pool(name="p1s", bufs=2))
                ps1 = pbatch.enter_context(tc.tile_pool(name="psum1", bufs=2, space="PSUM"))
                ps1d = pbatch.enter_context(tc.tile_pool(name="psum1d", bufs=1, space="PSUM"))

                for b in range(p1b):
                    xt = [p1x.tile([128, L], F32R, tag=f"xt{c}", name=f"xt{c}") for c in range(NDC)]
                    for c in range(NDC):
                        nc.sync.dma_start(
                            out=xt[c][:, :], in_=xT[b, 128 * c : 128 * (c + 1), :])

                    q_sb, k_sb = [], []
                    for lt in range(NLC):
                        pq = ps1.tile([128, D], F32, tag="pq")
                        pk = ps1.tile([128, D], F32, tag="pk")
                        for c in range(NDC):
                            nc.tensor.matmul(
                                pq[:, :], xt[c][:, 128 * lt : 128 * (lt + 1)],
                                wq_sb[c][:, :], start=(c == 0), stop=(c == NDC - 1))
                        for c in range(NDC):
                            nc.tensor.matmul(
                                pk[:, :], xt[c][:, 128 * lt : 128 * (lt + 1)],
                                wk_sb[c][:, :], start=(c == 0), stop=(c == NDC - 1))
                        qt = p1q.tile([128, D], F32R, tag=f"q{lt}")
                        kt = p1q.tile([128, D], F32R, tag=f"k{lt}")
                        nc.vector.tensor_add(qt[:, :], pq[:, :], bq_bc[:, :])
                        nc.vector.tensor_add(kt[:, :], pk[:, :], bk_bc[:, :])
                        q_sb.append(qt)
                        k_sb.append(kt)

                    for ft in range(p1ft):
                        m = min(128, LF - 128 * ft)
                        fsl = slice(128 * ft, 128 * ft + m)
                        pqr = ps1d.tile([128, D], F32, tag="pqr")
                        pqi = ps1d.tile([128, D], F32, tag="pqi")
                        pkr = ps1d.tile([128, D], F32, tag="pkr")
                        pki = ps1d.tile([128, D], F32, tag="pki")
                        for lc in range(NLC):
                            st_ = lc == 0
                            sp_ = lc == NLC - 1
                            nc.tensor.matmul(pqr[:m, :], cc_sb[lc][:, fsl],
                                             q_sb[lc][:, :], start=st_, stop=sp_)
                            nc.tensor.matmul(pqi[:m, :], cs_sb[lc][:, fsl],
                                             q_sb[lc][:, :], start=st_, stop=sp_)
                            nc.tensor.matmul(pkr[:m, :], cc_sb[lc][:, fsl],
                                             k_sb[lc][:, :], start=st_, stop=sp_)
                            nc.tensor.matmul(pki[:m, :], cs_sb[lc][:, fsl],
                                             k_sb[lc][:, :], start=st_, stop=sp_)
                        qr = p1e.tile([128, D], F32, tag="qr")
                        qi = p1e.tile([128, D], F32, tag="qi")
                        kr = p1e.tile([128, D], F32, tag="kr")
                        ki = p1e.tile([128, D], F32, tag="ki")
                        nc.scalar.copy(qr[:m, :], pqr[:m, :])
                        nc.scalar.copy(qi[:m, :], pqi[:m, :])
                        nc.scalar.copy(kr[:m, :], pkr[:m, :])
                        nc.scalar.copy(ki[:m, :], pki[:m, :])
                        scr = p1s.tile([128, D], F32, tag="scr")
                        acc = p1s.tile([128, 4], F32, tag="acc")
                        col = 5 * ft + b
                        prods = ((qr, kr, 0), (qi, ki, 1), (qi, kr, 2), (qr, ki, 3))
                        if SPROD_MODE == "stt":
                            for pa, pb, j in prods:
                                nc.vector.scalar_tensor_tensor(
                                    out=scr[:m, :], in0=pa[:m, :], scalar=1.0,
                                    in1=pb[:m, :], op0=OP.mult, op1=OP.mult,
                                    accum_out=acc[:m, j : j + 1])
                        else:
                            for pa, pb, j in prods:
                                nc.vector.tensor_mul(scr[:m, :], pa[:m, :], pb[:m, :])
                                nc.vector.reduce_sum(
                                    acc[:m, j : j + 1], scr[:m, :], axis=AX.X)
                        nc.vector.tensor_add(
                            s_re[:m, col : col + 1], acc[:m, 0:1], acc[:m, 1:2])
                        nc.vector.tensor_sub(
                            s_im[:m, col : col + 1], acc[:m, 2:3], acc[:m, 3:4])

                pbatch.close()
                # ================= PHASE 1.5: irfft, allreduce, topk, weights ======
                if dbg:
                    nc.sync.dma_start(out=s_dbg[:, 0 : 5 * NFT], in_=s_re[:, :])
                    nc.sync.dma_start(out=s_dbg[:, 5 * NFT : 10 * NFT], in_=s_im[:, :])
                if phases < 0:
                    nc.vector.memset(mv_sb[:, :], 0.0)
                    nc.vector.memset(idx_sb[:, :], 0)
                    nc.vector.memset(wbc[:, :], 0.0)
                    nc.vector.memset(wv[:, :], 0.0)
                nftr = range(NFT) if phases >= 0 else range(0)
                for ft in nftr:
                    nc.vector.reduce_sum(
                        s_re[:, 5 * ft + 4 : 5 * ft + 5], s_re[:, 5 * ft : 5 * ft + 4],
                        axis=AX.X)
                    nc.vector.reduce_sum(
                        s_im[:, 5 * ft + 4 : 5 * ft + 5], s_im[:, 5 * ft : 5 * ft + 4],
                        axis=AX.X)

                with (
                    tc.tile_pool(name="irf", bufs=3) as irf,
                    tc.tile_pool(name="psum15", bufs=2, space="PSUM") as ps15,
                    tc.tile_pool(name="psum15t", bufs=2, space="PSUM") as ps15t,
                ):
                    # Cc/Cs are still resident; derive the irfft operand
                    # (beta_f/(L*D) * [cos|-sin]^T) by transposing their tiles.
                    for nb in range(NLB) if phases >= 0 else range(0):
                        pmv = ps15.tile([5, 512], F32, tag="pmv")
                        first = True
                        for half, stile, csb in ((0, s_re, cc_sb), (1, s_im, cs_sb)):
                            for ft in range(NFT):
                                m = min(128, LF - 128 * ft)
                                fsl = slice(128 * ft, 128 * ft + m)
                                pt = ps15t.tile([128, 512], F32, tag="pt")
                                for j in range(4):
                                    lc = 4 * nb + j
                                    nc.tensor.transpose(
                                        pt[:m, 128 * j : 128 * (j + 1)],
                                        csb[lc][:, fsl].bitcast(F32),
                                        ident[:, :])
                                mt = irf.tile([128, 512], F32, tag="minv")
                                if m < 128:
                                    nc.vector.memset(mt[:, :], 0.0)
                                bcol = betac0 if (ft == 0 or ft == NFT - 1) else betac
                                nc.scalar.activation(
                                    mt[:m, :], pt[:m, :], AF.Copy, scale=bcol[:m, 0:1])
                                nc.tensor.matmul(
                                    pmv[:, :], stile[:, 5 * ft : 5 * ft + 5], mt[:, :],
                                    start=first, stop=(half == 1 and ft == NFT - 1))
                                first = False
                        nc.vector.tensor_copy(mv_sb[:, 512 * nb : 512 * (nb + 1)], pmv[:, :])

            # AllReduce the batch-summed spectrum row -> global over all 32 batches
            do_ar = phases >= 0 and kar != 0
            cc_in = dram.tile([1, L], F32)
            cc_out = dram.tile([1, L], F32)
            mvg = pp.tile([1, L], F32, tag="mvg")
            max8 = pp.tile([1, 8], F32, tag="max8")
            if phases >= 0 and not do_ar:
                nc.vector.memset(idx_sb[:, :], 0)
            if do_ar:
                nc.sync.dma_start(out=cc_in[:, :], in_=mv_sb[4:5, :])
                nc.gpsimd.collective_compute(
                    "AllReduce",
                    OP.add,
                    replica_groups=[list(range(N_CORES))],
                    ins=[cc_in[:, :].opt()],
                    outs=[cc_out[:, :].opt()],
                )
                nc.sync.dma_start(out=mvg[:, :], in_=cc_out[:, :])
                nc.vector.max(out=max8[:, :], in_=mvg[:, :])
                nc.vector.max_index(out=idx_sb[:, :], in_max=max8[:, :], in_values=mvg[:, :])

            ntk = range(TOPK) if phases >= 1 else range(0)
            dvals = [
                nc.values_load(
                    idx_sb[0:1, i : i + 1],
                    engines=OrderedSet([DVE]),
                    min_val=0,
                    max_val=L - 1,
                    skip_runtime_bounds_check=True,
                )
                for i in ntk
            ]

            # per-batch weights at the selected delays + softmax, then broadcast
            if 0 <= phases < 1:
                nc.vector.memset(wv[:, :], 0.0)
            for i in ntk:
                nc.vector.tensor_copy(wv[:, i : i + 1], mv_sb[0:BC, bass.ds(dvals[i], 1)])
            wred = pp.tile([BC, 2], F32, tag="wred")
            if phases < 1:
                nc.vector.memset(wbc[:, :], 0.0)
            if phases >= 1:
                nc.vector.reduce_max(wred[:, 0:1], wv[:, :], axis=AX.X)
                wexp = pp.tile([BC, TOPK], F32, tag="wexp")
                nc.vector.tensor_scalar(
                    wexp[:, :], wv[:, :], wred[:, 0:1], None, op0=OP.subtract)
                nc.scalar.activation(wexp[:, :], wexp[:, :], AF.Exp)
                nc.vector.reduce_sum(wred[:, 1:2], wexp[:, :], axis=AX.X)
                nc.vector.reciprocal(wred[:, 1:2], wred[:, 1:2])
                nc.vector.tensor_scalar(
                    wexp[:, :], wexp[:, :], wred[:, 1:2], None, op0=OP.mult)
                w_dram = dram.tile([BC, TOPK], F32)
                nc.sync.dma_start(out=w_dram[:, :], in_=wexp[:, :])
                wflat = pp.tile([1, BC * TOPK], F32, tag="wflat")
                nc.sync.dma_start(out=wflat[:, :], in_=w_dram[:, :])
                nc.gpsimd.partition_broadcast(wbc[:, :], wflat[0:1, :])

            if dbg:
                nc.sync.dma_start(out=mv_dbg[:, :], in_=mv_sb[:, :])
                nc.sync.dma_start(out=idx_dbg[:, :], in_=idx_sb[:, :])
                nc.sync.dma_start(out=w_dbg[:, :], in_=wexp[:, :] if phases >= 1 else wv[:, :])

            # ================= PHASE 2: rolls, decomp, FFN, decomp =============
            def ma_seasonal(pool, dst, src):
                """dst = src - moving_avg(src) along the free axis (edge-replicated).

                src must be F32-readable; dst may be F32 or F32R."""
                cs1 = pool.tile([128, L + 1], F32, tag="cs1", bufs=1)
                nc.vector.memset(cs1[:, 0:1], 0.0)
                nc.vector.tensor_tensor_scan(
                    cs1[:, 1 : L + 1], src[:, :], src[:, :], 0.0,
                    op0=OP.add, op1=OP.bypass)
                dif = pool.tile([128, L - 2 * PAD], F32, tag="dif", bufs=1)
                nc.vector.tensor_sub(
                    dif[:, :], cs1[:, 2 * PAD + 1 : L + 1], cs1[:, 0 : L - 2 * PAD])
                nc.vector.scalar_tensor_tensor(
                    out=dst[:, PAD : L - PAD], in0=dif[:, :], scalar=-1.0 / KMA,
                    in1=src[:, PAD : L - PAD], op0=OP.mult, op1=OP.add)
                # left edge: s[l] = x[l] - cs1[l+PAD+1]/K - (PAD-l)/K * x[0]
                nc.vector.scalar_tensor_tensor(
                    out=dst[:, 0:PAD], in0=cs1[:, PAD + 1 : 2 * PAD + 1],
                    scalar=-1.0 / KMA, in1=src[:, 0:PAD], op0=OP.mult, op1=OP.add)
                nc.vector.scalar_tensor_tensor(
                    out=dst[:, 0:PAD], in0=coefL_sb[:, :], scalar=src[:, 0:1],
                    in1=dst[:, 0:PAD], op0=OP.mult, op1=OP.add)
                # right edge: s[l] = x[l] - (stot - cs1[l-PAD])/K - (l-L+PAD+1)/K * x[L-1]
                e2 = pool.tile([128, PAD], F32, tag="e2", bufs=1)
                nc.vector.tensor_scalar(
                    e2[:, :], cs1[:, L - 2 * PAD : L - PAD], cs1[:, L : L + 1],
                    1.0 / KMA, op0=OP.subtract, op1=OP.mult)
                nc.vector.tensor_add(
                    dst[:, L - PAD : L], e2[:, :], src[:, L - PAD : L])
                nc.vector.scalar_tensor_tensor(
                    out=dst[:, L - PAD : L], in0=coefR_sb[:, :],
                    scalar=src[:, L - 1 : L], in1=dst[:, L - PAD : L],
                    op0=OP.mult, op1=OP.add)

            with ExitStack() as p2stack:
                nch = range(NDC) if phases >= 2 else range(0)
                nfh = range(NFC) if phases >= 2 else range(0)
                p2w = p2stack.enter_context(tc.tile_pool(name="p2w", bufs=1))
                w1_sb = []
                for c in nch:
                    t = p2w.tile([128, DFF], F32R, tag=f"w1_{c}")
                    nc.sync.dma_start(out=t[:, :], in_=W1_d[128 * c : 128 * (c + 1), :])
                    w1_sb.append(t)
                w2_sb = []
                for c in nfh:
                    t = p2w.tile([128, D], F32R, tag=f"w2_{c}")
                    nc.sync.dma_start(out=t[:, :], in_=W2_d[128 * c : 128 * (c + 1), :])
                    w2_sb.append(t)
                wvo_sb = []
                for c in nch:
                    t = p2w.tile([128, D], F32R, tag=f"wvo{c}")
                    nc.sync.dma_start(out=t[:, :], in_=Wvo_d[128 * c : 128 * (c + 1), :])
                    wvo_sb.append(t)

                p2x = p2stack.enter_context(tc.tile_pool(name="p2x", bufs=1))
                p2 = p2stack.enter_context(tc.tile_pool(name="p2", bufs=1))
                ps2 = p2stack.enter_context(tc.tile_pool(name="psum2", bufs=2, space="PSUM"))
                ps2y = p2stack.enter_context(tc.tile_pool(name="psum2y", bufs=1, space="PSUM"))

                for b in range(BC) if phases >= 2 else range(0):
                    xt = [p2x.tile([128, L], F32R, tag=f"x2t{c}", name=f"x2t{c}") for c in range(NDC)]
                    for c in range(NDC):
                        nc.sync.dma_start(
                            out=xt[c][:, :], in_=xT[b, 128 * c : 128 * (c + 1), :])

                    # vo' = x @ (Wv Wo); x2 = x + bvo + sum_i w_i roll(vo', d_i)
                    x2 = [p2.tile([128, L], F32, tag=f"x2_{c}", name=f"x2_{c}") for c in range(NDC)]
                    for c in range(NDC):
                        vo2 = p2.tile([128, 2 * L], F32, tag="vo2", bufs=1)
                        for nb in range(NLB):
                            pv = ps2.tile([128, 512], F32, tag="pv")
                            for cx in range(NDC):
                                nc.tensor.matmul(
                                    pv[:, :],
                                    wvo_sb[cx][:, 128 * c : 128 * (c + 1)],
                                    xt[cx][:, 512 * nb : 512 * (nb + 1)],
                                    start=(cx == 0),
                                    stop=(cx == NDC - 1),
                                )
                            nc.scalar.copy(vo2[:, 512 * nb : 512 * (nb + 1)], pv[:, :])
                            nc.scalar.copy(
                                vo2[:, L + 512 * nb : L + 512 * (nb + 1)], pv[:, :])
                        nc.scalar.activation(
                            x2[c][:, :], xt[c][:, :], AF.Identity,
                            bias=bvoT[:, c : c + 1])
                        for i in range(TOPK):
                            nc.vector.scalar_tensor_tensor(
                                out=x2[c][:, :],
                                in0=vo2[:, bass.ds(dvals[i], L)],
                                scalar=wbc[:, TOPK * b + i : TOPK * b + i + 1],
                                in1=x2[c][:, :],
                                op0=OP.mult,
                                op1=OP.add,
                            )

                    # first decomposition -> seasonal part s (f32r: feeds FFN)
                    st = [p2.tile([128, L], F32R, tag=f"st{c}", name=f"st{c}") for c in range(NDC)]
                    for c in range(NDC):
                        ma_seasonal(p2, st[c], x2[c])

                    # FFN + residual: z = s + relu(s W1 + b1) W2 + b2
                    z = x2  # reuse buffers
                    for nb in range(NLB):
                        lsl = slice(512 * nb, 512 * (nb + 1))
                        py = [ps2y.tile([128, 512], F32, tag=f"py{c}", name=f"py{c}") for c in range(NDC)]
                        for fc in range(NFC):
                            ph = ps2.tile([128, 512], F32, tag="ph")
                            for c in range(NDC):
                                nc.tensor.matmul(
                                    ph[:, :],
                                    w1_sb[c][:, 128 * fc : 128 * (fc + 1)],
                                    st[c][:, lsl],
                                    start=(c == 0),
                                    stop=(c == NDC - 1),
                                )
                            ht = p2.tile([128, 512], F32R, tag="ht", bufs=3)
                            nc.scalar.activation(
                                ht[:, :], ph[:, :], AF.Relu, bias=b1T[:, fc : fc + 1])
                            for c in range(NDC):
                                nc.tensor.matmul(
                                    py[c][:, :],
                                    w2_sb[fc][:, 128 * c : 128 * (c + 1)],
                                    ht[:, :],
                                    start=(fc == 0),
                                    stop=(fc == NFC - 1),
                                )
                        for c in range(NDC):
                            nc.vector.scalar_tensor_tensor(
                                out=z[c][:, lsl], in0=py[c][:, :],
                                scalar=b2T[:, c : c + 1], in1=st[c][:, lsl],
                                op0=OP.add, op1=OP.add)

                    # second decomposition -> output
                    for c in range(NDC):
                        rt = p2.tile([128, L], F32, tag="rt", bufs=1)
                        ma_seasonal(p2, rt, z[c])
                        nc.sync.dma_start(
                            out=resT[b, 128 * c : 128 * (c + 1), :], in_=rt[:, :])

    nc.compile()
    return nc


_CACHE = {}


def _get_nc(dbg=False):
    if dbg not in _CACHE:
        _CACHE[dbg] = build(dbg=dbg)
    return _CACHE[dbg]


def make_in_maps(x, Wq, bq, Wk, bk, Wv, bv, Wo, bo, W1, b1, W2, b2):
    x = np.asarray(x, np.float32)
    Wvo = (np.asarray(Wv, np.float64) @ np.asarray(Wo, np.float64)).astype(np.float32)
    bvo = (np.asarray(bv, np.float64) @ np.asarray(Wo, np.float64)
           + np.asarray(bo, np.float64)).astype(np.float32)
    shared = {
        "Wq": np.ascontiguousarray(Wq, np.float32),
        "Wk": np.ascontiguousarray(Wk, np.float32),
        "Wvo": Wvo,
        "W1": np.ascontiguousarray(W1, np.float32),
        "W2": np.ascontiguousarray(W2, np.float32),
        "bq": np.asarray(bq, np.float32).reshape(1, D),
        "bk": np.asarray(bk, np.float32).reshape(1, D),
        "bvo": np.ascontiguousarray(bvo.reshape(NDC, 128).T),
        "b1": np.ascontiguousarray(np.asarray(b1, np.float32).reshape(NFC, 128).T),
        "b2": np.ascontiguousarray(np.asarray(b2, np.float32).reshape(NDC, 128).T),
    }
    in_maps = []
    for c in range(N_CORES):
        xs = x[BC * c : BC * (c + 1)]
        in_maps.append({**shared, "xT": np.ascontiguousarray(xs.transpose(0, 2, 1))})
    return in_maps


class _JitRunner:
    """Holds a jitted shard_map for a compiled Bass module, so repeat calls
    skip retrace/relower/XLA-recompile (run_bass_kernel_spmd rebuilds the jit
    — and re-serializes the whole BIR into the custom call — on every call).

    Zero-init output buffers are created ON DEVICE (tiny jitted memset) and
    donated, instead of shipping host zeros every call.
    """

    def __init__(self, nc):
        import jax
        import jax.numpy as jnp
        from jax.sharding import Mesh, PartitionSpec, NamedSharding
        from jax.experimental.shard_map import shard_map
        from concourse import bass2jax

        bass2jax.install_neuronx_cc_hook()
        self.jax = jax
        self.nc = nc
        partition_name = (
            nc.partition_id_tensor.name if nc.partition_id_tensor else None)
        in_names, out_names, out_avals, out_shapes = [], [], [], []
        for alloc in nc.m.functions[0].allocations:
            if not isinstance(alloc, mybir.MemoryLocationSet):
                continue
            if alloc.kind == "ExternalInput":
                name = alloc.memorylocations[0].name
                if name != partition_name:
                    in_names.append(name)
            elif alloc.kind == "ExternalOutput":
                shape = tuple(alloc.tensor_shape)
                dtype = mybir.dt.np(alloc.dtype)
                out_avals.append(jax.core.ShapedArray(shape, dtype))
                out_names.append(alloc.memorylocations[0].name)
                out_shapes.append((shape, dtype))
        self.in_names = in_names
        self.out_names = out_names
        self.out_avals = out_avals
        n_params = len(in_names)
        n_outs = len(out_names)
        in_names_all = in_names + out_names
        if partition_name is not None:
            in_names_all.append(partition_name)

        def _body(*args):
            operands = list(args)
            if partition_name is not None:
                operands.append(bass2jax.partition_id_tensor())
            outs = bass2jax._bass_exec_p.bind(
                *operands,
                out_avals=tuple(out_avals),
                in_names=tuple(in_names_all),
                out_names=tuple(out_names),
                lowering_input_output_aliases=(),
                sim_require_finite=True,
                sim_require_nnan=True,
                nc=nc,
            )
            return tuple(outs)

        devices = jax.devices()[:N_CORES]
        mesh = Mesh(np.asarray(devices), ("core",))
        self.sharding = NamedSharding(mesh, PartitionSpec("core"))
        in_specs = (PartitionSpec("core"),) * (n_params + n_outs)
        out_specs = (PartitionSpec("core"),) * n_outs
        self.sharded = jax.jit(
            shard_map(_body, mesh=mesh, in_specs=in_specs,
                      out_specs=out_specs, check_rep=False),
            donate_argnums=tuple(range(n_params, n_params + n_outs)),
            keep_unused=True)

        def _mk_zeros():
            return tuple(
                jnp.zeros((N_CORES * s[0], *s[1:]), dt) for s, dt in out_shapes)

        self._zeros = jax.jit(
            _mk_zeros, out_shardings=(self.sharding,) * n_outs)

    def concat_inputs(self, in_maps):
        per_core = [[np.asarray(m[name]) for name in self.in_names]
                    for m in in_maps]
        return [np.concatenate([per_core[c][i] for c in range(N_CORES)], axis=0)
                for i in range(len(self.in_names))]

    def zeros(self):
        z = self._zeros()
        self.jax.block_until_ready(z)
        return z

    def call(self, args):
        outs = self.sharded(*args, *self.zeros())
        self.jax.block_until_ready(outs)
        return outs

    def run_np(self, in_maps):
        outs = self.call(self.concat_inputs(in_maps))
        return [
            {name: np.asarray(outs[i]).reshape(
                N_CORES, *self.out_avals[i].shape)[c]
             for i, name in enumerate(self.out_names)}
            for c in range(N_CORES)
        ]

    def time_resident(self, in_maps, reps=6):
        import time as _time
        jax = self.jax
        dev_in = [jax.device_put(a, self.sharding)
                  for a in self.concat_inputs(in_maps)]
        jax.block_until_ready(dev_in)
        self.call(dev_in)  # warmup (compile on first use)
        best = float("inf")
        for _ in range(reps):
            z = self.zeros()
            t0 = _time.time()
            outs = self.sharded(*dev_in, *z)
            jax.block_until_ready(outs)
            best = min(best, _time.time() - t0)
        return best


_RUNNERS = {}


def _get_runner(dbg=False):
    if dbg not in _RUNNERS:
        _RUNNERS[dbg] = _JitRunner(_get_nc(dbg=dbg))
    return _RUNNERS[dbg]


class _Res:
    def __init__(self, results):
        self.results = results
        self.exec_time_ns = None
        self.instructions_and_trace = None
        self.profile_json = None


def run(inputs, dbg=False, trace=False):
    runner = _get_runner(dbg=dbg)
    in_maps = make_in_maps(**inputs)
    results = runner.run_np(in_maps)
    out = np.empty((B, L, D), np.float32)
    for c in range(N_CORES):
        rt = results[c]["resT"]  # [BC, D, L]
        out[BC * c : BC * (c + 1)] = rt.transpose(0, 2, 1)
    return out, _Res(results)


def kernel(**inputs):
    out, _ = run(inputs)
    return out


_NULL_CACHE = {}


def _get_null_runner():
    if "r" not in _NULL_CACHE:
        nc = bacc.Bacc("TRN2", target_bir_lowering=False, debug=False,
                       num_devices=N_CORES)
        ins = {
            "xT": [BC, D, L], "Wq": [D, D], "Wk": [D, D], "Wvo": [D, D],
            "W1": [D, DFF], "W2": [DFF, D], "bq": [1, D], "bk": [1, D],
            "bvo": [128, NDC], "b1": [128, NFC], "b2": [128, NDC],
        }
        for name, shape in ins.items():
            nc.dram_tensor(name, shape, F32, kind="ExternalInput")
        resT = nc.dram_tensor("resT", [BC, D, L], F32, kind="ExternalOutput")
        with tile.TileContext(nc) as tc:
            with tc.tile_pool(name="sb", bufs=1) as sb:
                t = sb.tile([128, 4], F32, name="t")
                nc.vector.memset(t[:, :], 0.0)
                nc.sync.dma_start(out=resT[0, 0:128, 0:4], in_=t[:, :])
        nc.compile()
        _NULL_CACHE["r"] = _JitRunner(nc)
    return _NULL_CACHE["r"]


def time_null(inputs, reps=6):
    runner = _get_null_runner()
    in_maps = make_in_maps(**inputs)
    return runner.time_resident(in_maps, reps=reps)


def time_main(inputs, reps=6, dbg=False):
    runner = _get_runner(dbg=dbg)
    in_maps = make_in_maps(**inputs)
    return runner.time_resident(in_maps, reps=reps)



# revision 8
# speedup vs baseline: 427.7070x; 1.7289x over previous
"""Autoformer encoder layer on 8 Trainium2 NeuronCores (Bass/Tile).

Data-parallel over batch: each of the 8 cores processes 4 of the 32 batches.
Everything runs on-device in one NEFF, including the cross-core reduction for
the global top-k delay selection (AllReduce of the batch-summed correlation
spectrum) and the data-dependent delay rolls (dynamic-offset SBUF reads).

Math notes
----------
The reference only ever uses two reductions of the full per-(head,channel)
autocorrelation:
  * mean_value[b, l] = mean over channels of irfft(rfft(q)*conj(rfft(k)))
    -> computed here as S[b,f] = sum_c Qf*conj(Kf) via DFT-as-matmul over L,
       then one tiny irfft matmul.  (The full [B,H,E,L] corr is never needed.)
  * agg = sum_i softmax(w)_i * roll(v, -d_i)  -> rolls commute with the
    output projection, so v@Wo is computed directly with folded weights
    Wvo = Wv @ Wo and rolled instead (7 dynamic-slice MACs per channel chunk).
The moving-average decomposition runs as a cumsum scan along the free axis in
channel-major layout, so the whole residual/FFN pipeline needs no on-device
transposes: x is fed pre-transposed as [B, D, L] and the output is returned
transposed, undone on the host.
"""

import os
from contextlib import ExitStack

import numpy as np

import concourse.bass as bass
import concourse.bacc as bacc
import concourse.mybir as mybir
from concourse import tile
from concourse.bass_utils import run_bass_kernel_spmd
from concourse.ordered_set import OrderedSet

F32 = mybir.dt.float32
F32R = mybir.dt.float32r
U32 = mybir.dt.uint32
AX = mybir.AxisListType
OP = mybir.AluOpType
AF = mybir.ActivationFunctionType
DVE = mybir.EngineType.DVE
SPROD_MODE = os.environ.get("KSPROD", "stt")

B, L, D, DFF = 32, 1536, 512, 2048
KMA = 25              # moving-average window
PAD = (KMA - 1) // 2  # 12
TOPK = 7              # int(1 * log(1536))
N_CORES = 8
BC = B // N_CORES     # batches per core
LF = L // 2 + 1       # 769 rfft bins
NLC = L // 128        # 12 l-chunks
NDC = D // 128        # 4 channel chunks
NFC = DFF // 128      # 16 ffn chunks
NFT = (LF + 127) // 128  # 7 f-tiles (last has 1 row)
NLB = L // 512        # 3 l-blocks of 512


def _host_consts():
    lv = np.arange(L)
    fv = np.arange(LF)
    ang = 2.0 * np.pi * np.outer(lv, fv) / L
    Cc = np.cos(ang).astype(np.float32)           # [L, LF] rfft real basis
    Cs = (-np.sin(ang)).astype(np.float32)        # [L, LF] rfft imag basis
    # irfft with the channel-mean folded in:
    # mv[l] = (1/(L*D)) sum_f beta_f (Sre[f] cos(wfl) - Sim[f] sin(wfl))
    # moving-average edge coefficients, pre-negated for fused (coef*edge)+rest
    coefL = np.tile((-(PAD - np.arange(PAD)) / KMA).astype(np.float32), (128, 1))
    coefR = np.tile((-(np.arange(PAD) + 1) / KMA).astype(np.float32), (128, 1))
    return Cc, Cs, None, coefL, coefR


def build(dbg=False, kreps=None):
    phases = int(os.environ.get("KPHASES", "2"))
    p1ft = int(os.environ.get("KP1FT", str(NFT)))
    p1b = int(os.environ.get("KP1B", str(BC)))
    if kreps is None:
        kreps = int(os.environ.get("KREPS", "1"))
    kar = int(os.environ.get("KAR", "1"))
    nc = bacc.Bacc("TRN2", target_bir_lowering=False, debug=False, num_devices=N_CORES)

    xT = nc.dram_tensor("xT", [BC, D, L], F32R, kind="ExternalInput")
    Wq_d = nc.dram_tensor("Wq", [D, D], F32R, kind="ExternalInput")
    Wk_d = nc.dram_tensor("Wk", [D, D], F32R, kind="ExternalInput")
    Wvo_d = nc.dram_tensor("Wvo", [D, D], F32R, kind="ExternalInput")
    W1_d = nc.dram_tensor("W1", [D, DFF], F32R, kind="ExternalInput")
    W2_d = nc.dram_tensor("W2", [DFF, D], F32R, kind="ExternalInput")
    bq_d = nc.dram_tensor("bq", [1, D], F32, kind="ExternalInput")
    bk_d = nc.dram_tensor("bk", [1, D], F32, kind="ExternalInput")
    # channel-major biases prepacked host-side as [128, nchunks]
    bvo_d = nc.dram_tensor("bvo", [128, NDC], F32, kind="ExternalInput")
    b1_d = nc.dram_tensor("b1", [128, NFC], F32, kind="ExternalInput")
    b2_d = nc.dram_tensor("b2", [128, NDC], F32, kind="ExternalInput")
    # shape-derived constants: baked into the NEFF, no per-call transfer
    Cc_np, Cs_np, _, coefL_np, coefR_np = _host_consts()
    Cc_d = nc.inline_tensor(Cc_np, name="Cc")
    Cs_d = nc.inline_tensor(Cs_np, name="Cs")
    coefL_d = nc.inline_tensor(coefL_np, name="coefL")
    coefR_d = nc.inline_tensor(coefR_np, name="coefR")

    resT = nc.dram_tensor("resT", [BC, D, L], F32, kind="ExternalOutput")
    if dbg:
        s_dbg = nc.dram_tensor("s_dbg", [128, 10 * NFT], F32, kind="ExternalOutput")
        mv_dbg = nc.dram_tensor("mv_dbg", [5, L], F32, kind="ExternalOutput")
        idx_dbg = nc.dram_tensor("idx_dbg", [1, 8], U32, kind="ExternalOutput")
        w_dbg = nc.dram_tensor("w_dbg", [BC, TOPK], F32, kind="ExternalOutput")

    with tile.TileContext(nc) as tc, ExitStack() as stack:
        pp = stack.enter_context(tc.tile_pool(name="persist", bufs=1))
        dram = stack.enter_context(tc.tile_pool(name="dram", bufs=1, space="DRAM"))

        # ---- persistent biases --------------------------------------------
        bq_bc = pp.tile([128, D], F32, tag="bqbc")
        bk_bc = pp.tile([128, D], F32, tag="bkbc")
        brow = pp.tile([1, D], F32, tag="brow")
        nc.sync.dma_start(out=brow[:, :], in_=bq_d[:, :])
        nc.gpsimd.partition_broadcast(bq_bc[:, :], brow[0:1, :])
        brow2 = pp.tile([1, D], F32, tag="brow2")
        nc.sync.dma_start(out=brow2[:, :], in_=bk_d[:, :])
        nc.gpsimd.partition_broadcast(bk_bc[:, :], brow2[0:1, :])

        bvoT = pp.tile([128, NDC], F32, tag="bvoT")
        b1T = pp.tile([128, NFC], F32, tag="b1T")
        b2T = pp.tile([128, NDC], F32, tag="b2T")
        nc.sync.dma_start(out=bvoT[:, :], in_=bvo_d[:, :])
        nc.sync.dma_start(out=b1T[:, :], in_=b1_d[:, :])
        nc.sync.dma_start(out=b2T[:, :], in_=b2_d[:, :])

        coefL_sb = pp.tile([128, PAD], F32, tag="coefL")
        coefR_sb = pp.tile([128, PAD], F32, tag="coefR")
        nc.sync.dma_start(out=coefL_sb[:, :], in_=coefL_d[:, :])
        nc.sync.dma_start(out=coefR_sb[:, :], in_=coefR_d[:, :])

        # identity matrix for PE transposes + irfft row scales
        idint = pp.tile([128, 128], mybir.dt.int32, tag="idint")
        nc.gpsimd.iota(idint[:, :], pattern=[[1, 128]], base=0, channel_multiplier=-1)
        ident = pp.tile([128, 128], F32, tag="ident")
        nc.vector.tensor_scalar(ident[:, :], idint[:, :], 0, None, op0=OP.is_equal)
        betac = pp.tile([128, 1], F32, tag="betac")
        nc.vector.memset(betac[:, :], 2.0 / (L * D))
        betac0 = pp.tile([128, 1], F32, tag="betac0")
        nc.vector.memset(betac0[:, :], 2.0 / (L * D))
        nc.vector.memset(betac0[0:1, :], 1.0 / (L * D))

        # spectrum accumulator: 7 f-tiles x (4 batches + batchsum) columns
        s_re = pp.tile([128, 5 * NFT], F32, tag="s_re")
        s_im = pp.tile([128, 5 * NFT], F32, tag="s_im")
        nc.vector.memset(s_re[:, :], 0.0)
        nc.vector.memset(s_im[:, :], 0.0)

        mv_sb = pp.tile([5, L], F32, tag="mv")  # rows 0-3: batches, 4: batchsum
        idx_sb = pp.tile([1, 8], U32, tag="idx")
        wbc = pp.tile([128, BC * TOPK], F32, tag="wbc")
        wv = pp.tile([BC, TOPK], F32, tag="wv")

        for _rep in range(kreps):
            # ================= PHASE 1: q/k projections + DFT spectrum =========
            with ExitStack() as p1stack:
                p1c = p1stack.enter_context(tc.tile_pool(name="p1c", bufs=1))
                wq_sb, wk_sb = [], []
                for c in range(NDC):
                    t = p1c.tile([128, D], F32R, tag=f"wq{c}")
                    nc.sync.dma_start(out=t[:, :], in_=Wq_d[128 * c : 128 * (c + 1), :])
                    wq_sb.append(t)
                    t = p1c.tile([128, D], F32R, tag=f"wk{c}")
                    nc.sync.dma_start(out=t[:, :], in_=Wk_d[128 * c : 128 * (c + 1), :])
                    wk_sb.append(t)
                cc_sb, cs_sb = [], []
                for lc in range(NLC):
                    t = p1c.tile([128, LF], F32R, tag=f"cc{lc}")
                    nc.sync.dma_start(
                        out=t[:, :],
                        in_=Cc_d[128 * lc : 128 * (lc + 1), :].bitcast(F32R))
                    cc_sb.append(t)
                    t = p1c.tile([128, LF], F32R, tag=f"cs{lc}")
                    nc.sync.dma_start(
                        out=t[:, :],
                        in_=Cs_d[128 * lc : 128 * (lc + 1), :].bitcast(F32R))
                    cs_sb.append(t)

                pbatch = ExitStack()
                p1x = pbatch.enter_context(tc.tile_pool(name="p1x", bufs=1))
                p1q = pbatch.enter_context(tc.tile_pool(name="p1q", bufs=1))
                p1e = pbatch.enter_context(tc.tile_pool(name="p1e", bufs=1))
                p1s = pbatch.enter_context(tc.tile_pool(name="p1s", bufs=2))
                ps1 = pbatch.enter_context(tc.tile_pool(name="psum1", bufs=2, space="PSUM"))
                ps1d = pbatch.enter_context(tc.tile_pool(name="psum1d", bufs=1, space="PSUM"))

                for b in range(p1b):
                    xt = [p1x.tile([128, L], F32R, tag=f"xt{c}", name=f"xt{c}") for c in range(NDC)]
                    for c in range(NDC):
                        nc.sync.dma_start(
                            out=xt[c][:, :], in_=xT[b, 128 * c : 128 * (c + 1), :])

                    q_sb, k_sb = [], []
                    for lt in range(NLC):
                        pq = ps1.tile([128, D], F32, tag="pq")
                        pk = ps1.tile([128, D], F32, tag="pk")
                        for c in range(NDC):
                            nc.tensor.matmul(
                                pq[:, :], xt[c][:, 128 * lt : 128 * (lt + 1)],
                                wq_sb[c][:, :], start=(c == 0), stop=(c == NDC - 1))
                        for c in range(NDC):
                            nc.tensor.matmul(
                                pk[:, :], xt[c][:, 128 * lt : 128 * (lt + 1)],
                                wk_sb[c][:, :], start=(c == 0), stop=(c == NDC - 1))
                        qt = p1q.tile([128, D], F32R, tag=f"q{lt}")
                        kt = p1q.tile([128, D], F32R, tag=f"k{lt}")
                        nc.vector.tensor_add(qt[:, :], pq[:, :], bq_bc[:, :])
                        nc.vector.tensor_add(kt[:, :], pk[:, :], bk_bc[:, :])
                        q_sb.append(qt)
                        k_sb.append(kt)

                    for ft in range(p1ft):
                        m = min(128, LF - 128 * ft)
                        fsl = slice(128 * ft, 128 * ft + m)
                        pqr = ps1d.tile([128, D], F32, tag="pqr")
                        pqi = ps1d.tile([128, D], F32, tag="pqi")
                        pkr = ps1d.tile([128, D], F32, tag="pkr")
                        pki = ps1d.tile([128, D], F32, tag="pki")
                        for lc in range(NLC):
                            st_ = lc == 0
                            sp_ = lc == NLC - 1
                            nc.tensor.matmul(pqr[:m, :], cc_sb[lc][:, fsl],
                                             q_sb[lc][:, :], start=st_, stop=sp_)
                            nc.tensor.matmul(pqi[:m, :], cs_sb[lc][:, fsl],
                                             q_sb[lc][:, :], start=st_, stop=sp_)
                            nc.tensor.matmul(pkr[:m, :], cc_sb[lc][:, fsl],
                                             k_sb[lc][:, :], start=st_, stop=sp_)
                            nc.tensor.matmul(pki[:m, :], cs_sb[lc][:, fsl],
                                             k_sb[lc][:, :], start=st_, stop=sp_)
                        qr = p1e.tile([128, D], F32, tag="qr")
                        qi = p1e.tile([128, D], F32, tag="qi")
                        kr = p1e.tile([128, D], F32, tag="kr")
                        ki = p1e.tile([128, D], F32, tag="ki")
                        nc.scalar.copy(qr[:m, :], pqr[:m, :])
                        nc.scalar.copy(qi[:m, :], pqi[:m, :])
                        nc.scalar.copy(kr[:m, :], pkr[:m, :])
                        nc.scalar.copy(ki[:m, :], pki[:m, :])
                        scr = p1s.tile([128, D], F32, tag="scr")
                        acc = p1s.tile([128, 4], F32, tag="acc")
                        col = 5 * ft + b
                        prods = ((qr, kr, 0), (qi, ki, 1), (qi, kr, 2), (qr, ki, 3))
                        if SPROD_MODE == "stt":
                            for pa, pb, j in prods:
                                nc.vector.scalar_tensor_tensor(
                                    out=scr[:m, :], in0=pa[:m, :], scalar=1.0,
                                    in1=pb[:m, :], op0=OP.mult, op1=OP.mult,
                                    accum_out=acc[:m, j : j + 1])
                        else:
                            for pa, pb, j in prods:
                                nc.vector.tensor_mul(scr[:m, :], pa[:m, :], pb[:m, :])
                                nc.vector.reduce_sum(
                                    acc[:m, j : j + 1], scr[:m, :], axis=AX.X)
                        nc.vector.tensor_add(
                            s_re[:m, col : col + 1], acc[:m, 0:1], acc[:m, 1:2])
                        nc.vector.tensor_sub(
                            s_im[:m, col : col + 1], acc[:m, 2:3], acc[:m, 3:4])

                pbatch.close()
                # ================= PHASE 1.5: irfft, allreduce, topk, weights ======
                if dbg:
                    nc.sync.dma_start(out=s_dbg[:, 0 : 5 * NFT], in_=s_re[:, :])
                    nc.sync.dma_start(out=s_dbg[:, 5 * NFT : 10 * NFT], in_=s_im[:, :])
                if phases < 0:
                    nc.vector.memset(mv_sb[:, :], 0.0)
                    nc.vector.memset(idx_sb[:, :], 0)
                    nc.vector.memset(wbc[:, :], 0.0)
                    nc.vector.memset(wv[:, :], 0.0)
                nftr = range(NFT) if phases >= 0 else range(0)
                for ft in nftr:
                    nc.vector.reduce_sum(
                        s_re[:, 5 * ft + 4 : 5 * ft + 5], s_re[:, 5 * ft : 5 * ft + 4],
                        axis=AX.X)
                    nc.vector.reduce_sum(
                        s_im[:, 5 * ft + 4 : 5 * ft + 5], s_im[:, 5 * ft : 5 * ft + 4],
                        axis=AX.X)

                with (
                    tc.tile_pool(name="irf", bufs=3) as irf,
                    tc.tile_pool(name="psum15", bufs=2, space="PSUM") as ps15,
                    tc.tile_pool(name="psum15t", bufs=2, space="PSUM") as ps15t,
                ):
                    # Cc/Cs are still resident; derive the irfft operand
                    # (beta_f/(L*D) * [cos|-sin]^T) by transposing their tiles.
                    for nb in range(NLB) if phases >= 0 else range(0):
                        pmv = ps15.tile([5, 512], F32, tag="pmv")
                        first = True
                        for half, stile, csb in ((0, s_re, cc_sb), (1, s_im, cs_sb)):
                            for ft in range(NFT):
                                m = min(128, LF - 128 * ft)
                                fsl = slice(128 * ft, 128 * ft + m)
                                pt = ps15t.tile([128, 512], F32, tag="pt")
                                for j in range(4):
                                    lc = 4 * nb + j
                                    nc.tensor.transpose(
                                        pt[:m, 128 * j : 128 * (j + 1)],
                                        csb[lc][:, fsl].bitcast(F32),
                                        ident[:, :])
                                mt = irf.tile([128, 512], F32, tag="minv")
                                if m < 128:
                                    nc.vector.memset(mt[:, :], 0.0)
                                bcol = betac0 if (ft == 0 or ft == NFT - 1) else betac
                                nc.scalar.activation(
                                    mt[:m, :], pt[:m, :], AF.Copy, scale=bcol[:m, 0:1])
                                nc.tensor.matmul(
                                    pmv[:, :], stile[:, 5 * ft : 5 * ft + 5], mt[:, :],
                                    start=first, stop=(half == 1 and ft == NFT - 1))
                                first = False
                        nc.vector.tensor_copy(mv_sb[:, 512 * nb : 512 * (nb + 1)], pmv[:, :])

            # AllReduce the batch-summed spectrum row -> global over all 32 batches
            do_ar = phases >= 0 and kar != 0
            cc_in = dram.tile([1, L], F32)
            cc_out = dram.tile([1, L], F32)
            mvg = pp.tile([1, L], F32, tag="mvg")
            max8 = pp.tile([1, 8], F32, tag="max8")
            if phases >= 0 and not do_ar:
                nc.vector.memset(idx_sb[:, :], 0)
            if do_ar:
                nc.sync.dma_start(out=cc_in[:, :], in_=mv_sb[4:5, :])
                nc.gpsimd.collective_compute(
                    "AllReduce",
                    OP.add,
                    replica_groups=[list(range(N_CORES))],
                    ins=[cc_in[:, :].opt()],
                    outs=[cc_out[:, :].opt()],
                )
                nc.sync.dma_start(out=mvg[:, :], in_=cc_out[:, :])
                nc.vector.max(out=max8[:, :], in_=mvg[:, :])
                nc.vector.max_index(out=idx_sb[:, :], in_max=max8[:, :], in_values=mvg[:, :])

            ntk = range(TOPK) if phases >= 1 else range(0)
            dvals = [
                nc.values_load(
                    idx_sb[0:1, i : i + 1],
                    engines=OrderedSet([DVE]),
                    min_val=0,
                    max_val=L - 1,
                    skip_runtime_bounds_check=True,
                )
                for i in ntk
            ]

            # per-batch weights at the selected delays + softmax, then broadcast
            if 0 <= phases < 1:
                nc.vector.memset(wv[:, :], 0.0)
            for i in ntk:
                nc.vector.tensor_copy(wv[:, i : i + 1], mv_sb[0:BC, bass.ds(dvals[i], 1)])
            wred = pp.tile([BC, 2], F32, tag="wred")
            if phases < 1:
                nc.vector.memset(wbc[:, :], 0.0)
            if phases >= 1:
                nc.vector.reduce_max(wred[:, 0:1], wv[:, :], axis=AX.X)
                wexp = pp.tile([BC, TOPK], F32, tag="wexp")
                nc.vector.tensor_scalar(
                    wexp[:, :], wv[:, :], wred[:, 0:1], None, op0=OP.subtract)
                nc.scalar.activation(wexp[:, :], wexp[:, :], AF.Exp)
                nc.vector.reduce_sum(wred[:, 1:2], wexp[:, :], axis=AX.X)
                nc.vector.reciprocal(wred[:, 1:2], wred[:, 1:2])
                nc.vector.tensor_scalar(
                    wexp[:, :], wexp[:, :], wred[:, 1:2], None, op0=OP.mult)
                w_dram = dram.tile([BC, TOPK], F32)
                nc.sync.dma_start(out=w_dram[:, :], in_=wexp[:, :])
                wflat = pp.tile([1, BC * TOPK], F32, tag="wflat")
                nc.sync.dma_start(out=wflat[:, :], in_=w_dram[:, :])
                nc.gpsimd.partition_broadcast(wbc[:, :], wflat[0:1, :])

            if dbg:
                nc.sync.dma_start(out=mv_dbg[:, :], in_=mv_sb[:, :])
                nc.sync.dma_start(out=idx_dbg[:, :], in_=idx_sb[:, :])
                nc.sync.dma_start(out=w_dbg[:, :], in_=wexp[:, :] if phases >= 1 else wv[:, :])

            # ================= PHASE 2: rolls, decomp, FFN, decomp =============
            def ma_seasonal(pool, dst, src):
                """dst = src - moving_avg(src) along the free axis (edge-replicated).

                src must be F32-readable; dst may be F32 or F32R."""
                cs1 = pool.tile([128, L + 1], F32, tag="cs1", bufs=1)
                nc.vector.memset(cs1[:, 0:1], 0.0)
                nc.vector.tensor_tensor_scan(
                    cs1[:, 1 : L + 1], src[:, :], src[:, :], 0.0,
                    op0=OP.add, op1=OP.bypass)
                dif = pool.tile([128, L - 2 * PAD], F32, tag="dif", bufs=1)
                nc.vector.tensor_sub(
                    dif[:, :], cs1[:, 2 * PAD + 1 : L + 1], cs1[:, 0 : L - 2 * PAD])
                nc.vector.scalar_tensor_tensor(
                    out=dst[:, PAD : L - PAD], in0=dif[:, :], scalar=-1.0 / KMA,
                    in1=src[:, PAD : L - PAD], op0=OP.mult, op1=OP.add)
                # left edge: s[l] = x[l] - cs1[l+PAD+1]/K - (PAD-l)/K * x[0]
                nc.vector.scalar_tensor_tensor(
                    out=dst[:, 0:PAD], in0=cs1[:, PAD + 1 : 2 * PAD + 1],
                    scalar=-1.0 / KMA, in1=src[:, 0:PAD], op0=OP.mult, op1=OP.add)
                nc.vector.scalar_tensor_tensor(
                    out=dst[:, 0:PAD], in0=coefL_sb[:, :], scalar=src[:, 0:1],
                    in1=dst[:, 0:PAD], op0=OP.mult, op1=OP.add)
                # right edge: s[l] = x[l] - (stot - cs1[l-PAD])/K - (l-L+PAD+1)/K * x[L-1]
                e2 = pool.tile([128, PAD], F32, tag="e2", bufs=1)
                nc.vector.tensor_scalar(
                    e2[:, :], cs1[:, L - 2 * PAD : L - PAD], cs1[:, L : L + 1],
                    1.0 / KMA, op0=OP.subtract, op1=OP.mult)
                nc.vector.tensor_add(
                    dst[:, L - PAD : L], e2[:, :], src[:, L - PAD : L])
                nc.vector.scalar_tensor_tensor(
                    out=dst[:, L - PAD : L], in0=coefR_sb[:, :],
                    scalar=src[:, L - 1 : L], in1=dst[:, L - PAD : L],
                    op0=OP.mult, op1=OP.add)

            with ExitStack() as p2stack:
                nch = range(NDC) if phases >= 2 else range(0)
                nfh = range(NFC) if phases >= 2 else range(0)
                p2w = p2stack.enter_context(tc.tile_pool(name="p2w", bufs=1))
                w1_sb = []
                for c in nch:
                    t = p2w.tile([128, DFF], F32R, tag=f"w1_{c}")
                    nc.sync.dma_start(out=t[:, :], in_=W1_d[128 * c : 128 * (c + 1), :])
                    w1_sb.append(t)
                w2_sb = []
                for c in nfh:
                    t = p2w.tile([128, D], F32R, tag=f"w2_{c}")
                    nc.sync.dma_start(out=t[:, :], in_=W2_d[128 * c : 128 * (c + 1), :])
                    w2_sb.append(t)
                wvo_sb = []
                for c in nch:
                    t = p2w.tile([128, D], F32R, tag=f"wvo{c}")
                    nc.sync.dma_start(out=t[:, :], in_=Wvo_d[128 * c : 128 * (c + 1), :])
                    wvo_sb.append(t)

                p2x = p2stack.enter_context(tc.tile_pool(name="p2x", bufs=1))
                p2 = p2stack.enter_context(tc.tile_pool(name="p2", bufs=1))
                ps2 = p2stack.enter_context(tc.tile_pool(name="psum2", bufs=2, space="PSUM"))
                ps2y = p2stack.enter_context(tc.tile_pool(name="psum2y", bufs=1, space="PSUM"))

                for b in range(BC) if phases >= 2 else range(0):
                    xt = [p2x.tile([128, L], F32R, tag=f"x2t{c}", name=f"x2t{c}") for c in range(NDC)]
                    for c in range(NDC):
                        nc.sync.dma_start(
                            out=xt[c][:, :], in_=xT[b, 128 * c : 128 * (c + 1), :])

                    # vo' = x @ (Wv Wo); x2 = x + bvo + sum_i w_i roll(vo', d_i)
                    x2 = [p2.tile([128, L], F32, tag=f"x2_{c}", name=f"x2_{c}") for c in range(NDC)]
                    for c in range(NDC):
                        vo2 = p2.tile([128, 2 * L], F32, tag="vo2", bufs=1)
                        for nb in range(NLB):
                            pv = ps2.tile([128, 512], F32, tag="pv")
                            for cx in range(NDC):
                                nc.tensor.matmul(
                                    pv[:, :],
                                    wvo_sb[cx][:, 128 * c : 128 * (c + 1)],
                                    xt[cx][:, 512 * nb : 512 * (nb + 1)],
                                    start=(cx == 0),
                                    stop=(cx == NDC - 1),
                                )
                            nc.scalar.copy(vo2[:, 512 * nb : 512 * (nb + 1)], pv[:, :])
                            nc.scalar.copy(
                                vo2[:, L + 512 * nb : L + 512 * (nb + 1)], pv[:, :])
                        nc.scalar.activation(
                            x2[c][:, :], xt[c][:, :], AF.Identity,
                            bias=bvoT[:, c : c + 1])
                        for i in range(TOPK):
                            nc.vector.scalar_tensor_tensor(
                                out=x2[c][:, :],
                                in0=vo2[:, bass.ds(dvals[i], L)],
                                scalar=wbc[:, TOPK * b + i : TOPK * b + i + 1],
                                in1=x2[c][:, :],
                                op0=OP.mult,
                                op1=OP.add,
                            )

                    # first decomposition -> seasonal part s (f32r: feeds FFN)
                    st = [p2.tile([128, L], F32R, tag=f"st{c}", name=f"st{c}") for c in range(NDC)]
                    for c in range(NDC):
                        ma_seasonal(p2, st[c], x2[c])

                    # FFN + residual: z = s + relu(s W1 + b1) W2 + b2
                    z = x2  # reuse buffers
                    for nb in range(NLB):
                        lsl = slice(512 * nb, 512 * (nb + 1))
                        py = [ps2y.tile([128, 512], F32, tag=f"py{c}", name=f"py{c}") for c in range(NDC)]
                        for fc in range(NFC):
                            ph = ps2.tile([128, 512], F32, tag="ph")
                            for c in range(NDC):
                                nc.tensor.matmul(
                                    ph[:, :],
                                    w1_sb[c][:, 128 * fc : 128 * (fc + 1)],
                                    st[c][:, lsl],
                                    start=(c == 0),
                                    stop=(c == NDC - 1),
                                )
                            ht = p2.tile([128, 512], F32R, tag="ht", bufs=3)
                            nc.scalar.activation(
                                ht[:, :], ph[:, :], AF.Relu, bias=b1T[:, fc : fc + 1])
                            for c in range(NDC):
                                nc.tensor.matmul(
                                    py[c][:, :],
                                    w2_sb[fc][:, 128 * c : 128 * (c + 1)],
                                    ht[:, :],
                                    start=(fc == 0),
                                    stop=(fc == NFC - 1),
                                )
                        for c in range(NDC):
                            nc.vector.scalar_tensor_tensor(
                                out=z[c][:, lsl], in0=py[c][:, :],
                                scalar=b2T[:, c : c + 1], in1=st[c][:, lsl],
                                op0=OP.add, op1=OP.add)

                    # second decomposition -> output
                    for c in range(NDC):
                        rt = p2.tile([128, L], F32, tag="rt", bufs=1)
                        ma_seasonal(p2, rt, z[c])
                        nc.sync.dma_start(
                            out=resT[b, 128 * c : 128 * (c + 1), :], in_=rt[:, :])

    nc.compile()
    return nc


_CACHE = {}


def _get_nc(dbg=False, kreps=None):
    key = (dbg, kreps)
    if key not in _CACHE:
        _CACHE[key] = build(dbg=dbg, kreps=kreps)
    return _CACHE[key]


def make_in_maps(x, Wq, bq, Wk, bk, Wv, bv, Wo, bo, W1, b1, W2, b2):
    x = np.asarray(x, np.float32)
    Wvo = (np.asarray(Wv, np.float64) @ np.asarray(Wo, np.float64)).astype(np.float32)
    bvo = (np.asarray(bv, np.float64) @ np.asarray(Wo, np.float64)
           + np.asarray(bo, np.float64)).astype(np.float32)
    shared = {
        "Wq": np.ascontiguousarray(Wq, np.float32),
        "Wk": np.ascontiguousarray(Wk, np.float32),
        "Wvo": Wvo,
        "W1": np.ascontiguousarray(W1, np.float32),
        "W2": np.ascontiguousarray(W2, np.float32),
        "bq": np.asarray(bq, np.float32).reshape(1, D),
        "bk": np.asarray(bk, np.float32).reshape(1, D),
        "bvo": np.ascontiguousarray(bvo.reshape(NDC, 128).T),
        "b1": np.ascontiguousarray(np.asarray(b1, np.float32).reshape(NFC, 128).T),
        "b2": np.ascontiguousarray(np.asarray(b2, np.float32).reshape(NDC, 128).T),
    }
    in_maps = []
    for c in range(N_CORES):
        xs = x[BC * c : BC * (c + 1)]
        in_maps.append({**shared, "xT": np.ascontiguousarray(xs.transpose(0, 2, 1))})
    return in_maps


class _JitRunner:
    """Holds a jitted shard_map for a compiled Bass module, so repeat calls
    skip retrace/relower/XLA-recompile (run_bass_kernel_spmd rebuilds the jit
    — and re-serializes the whole BIR into the custom call — on every call).

    Zero-init output buffers are created ON DEVICE (tiny jitted memset) and
    donated, instead of shipping host zeros every call.
    """

    def __init__(self, nc):
        import jax
        import jax.numpy as jnp
        from jax.sharding import Mesh, PartitionSpec, NamedSharding
        from jax.experimental.shard_map import shard_map
        from concourse import bass2jax

        bass2jax.install_neuronx_cc_hook()
        self.jax = jax
        self.nc = nc
        partition_name = (
            nc.partition_id_tensor.name if nc.partition_id_tensor else None)
        in_names, out_names, out_avals, out_shapes = [], [], [], []
        for alloc in nc.m.functions[0].allocations:
            if not isinstance(alloc, mybir.MemoryLocationSet):
                continue
            if alloc.kind == "ExternalInput":
                name = alloc.memorylocations[0].name
                if name != partition_name:
                    in_names.append(name)
            elif alloc.kind == "ExternalOutput":
                shape = tuple(alloc.tensor_shape)
                dtype = mybir.dt.np(alloc.dtype)
                out_avals.append(jax.core.ShapedArray(shape, dtype))
                out_names.append(alloc.memorylocations[0].name)
                out_shapes.append((shape, dtype))
        self.in_names = in_names
        self.out_names = out_names
        self.out_avals = out_avals
        n_params = len(in_names)
        n_outs = len(out_names)
        in_names_all = in_names + out_names
        if partition_name is not None:
            in_names_all.append(partition_name)

        def _body(*args):
            operands = list(args)
            if partition_name is not None:
                operands.append(bass2jax.partition_id_tensor())
            outs = bass2jax._bass_exec_p.bind(
                *operands,
                out_avals=tuple(out_avals),
                in_names=tuple(in_names_all),
                out_names=tuple(out_names),
                lowering_input_output_aliases=(),
                sim_require_finite=True,
                sim_require_nnan=True,
                nc=nc,
            )
            return tuple(outs)

        devices = jax.devices()[:N_CORES]
        mesh = Mesh(np.asarray(devices), ("core",))
        self.sharding = NamedSharding(mesh, PartitionSpec("core"))
        in_specs = (PartitionSpec("core"),) * (n_params + n_outs)
        out_specs = (PartitionSpec("core"),) * n_outs
        self.sharded = jax.jit(
            shard_map(_body, mesh=mesh, in_specs=in_specs,
                      out_specs=out_specs, check_rep=False),
            donate_argnums=tuple(range(n_params, n_params + n_outs)),
            keep_unused=True)

        def _mk_zeros():
            return tuple(
                jnp.zeros((N_CORES * s[0], *s[1:]), dt) for s, dt in out_shapes)

        self._zeros = jax.jit(
            _mk_zeros, out_shardings=(self.sharding,) * n_outs)

    def concat_inputs(self, in_maps):
        per_core = [[np.asarray(m[name]) for name in self.in_names]
                    for m in in_maps]
        return [np.concatenate([per_core[c][i] for c in range(N_CORES)], axis=0)
                for i in range(len(self.in_names))]

    def zeros(self):
        z = self._zeros()
        self.jax.block_until_ready(z)
        return z

    def call(self, args):
        outs = self.sharded(*args, *self.zeros())
        self.jax.block_until_ready(outs)
        return outs

    def run_np(self, in_maps):
        outs = self.call(self.concat_inputs(in_maps))
        return [
            {name: np.asarray(outs[i]).reshape(
                N_CORES, *self.out_avals[i].shape)[c]
             for i, name in enumerate(self.out_names)}
            for c in range(N_CORES)
        ]

    def time_resident(self, in_maps, reps=6):
        import time as _time
        jax = self.jax
        dev_in = [jax.device_put(a, self.sharding)
                  for a in self.concat_inputs(in_maps)]
        jax.block_until_ready(dev_in)
        self.call(dev_in)  # warmup (compile on first use)
        best = float("inf")
        for _ in range(reps):
            z = self.zeros()
            t0 = _time.time()
            outs = self.sharded(*dev_in, *z)
            jax.block_until_ready(outs)
            best = min(best, _time.time() - t0)
        return best


_RUNNERS = {}


def _get_runner(dbg=False, kreps=None):
    key = (dbg, kreps)
    if key not in _RUNNERS:
        _RUNNERS[key] = _JitRunner(_get_nc(dbg=dbg, kreps=kreps))
    return _RUNNERS[key]


class _Res:
    def __init__(self, results):
        self.results = results
        self.exec_time_ns = None
        self.instructions_and_trace = None
        self.profile_json = None


def run(inputs, dbg=False, trace=False):
    runner = _get_runner(dbg=dbg)
    in_maps = make_in_maps(**inputs)
    results = runner.run_np(in_maps)
    out = np.empty((B, L, D), np.float32)
    for c in range(N_CORES):
        rt = results[c]["resT"]  # [BC, D, L]
        out[BC * c : BC * (c + 1)] = rt.transpose(0, 2, 1)
    return out, _Res(results)


def kernel(**inputs):
    out, _ = run(inputs)
    return out


_NULL_CACHE = {}


def _get_null_runner():
    if "r" not in _NULL_CACHE:
        nc = bacc.Bacc("TRN2", target_bir_lowering=False, debug=False,
                       num_devices=N_CORES)
        ins = {
            "xT": [BC, D, L], "Wq": [D, D], "Wk": [D, D], "Wvo": [D, D],
            "W1": [D, DFF], "W2": [DFF, D], "bq": [1, D], "bk": [1, D],
            "bvo": [128, NDC], "b1": [128, NFC], "b2": [128, NDC],
        }
        for name, shape in ins.items():
            nc.dram_tensor(name, shape, F32, kind="ExternalInput")
        resT = nc.dram_tensor("resT", [BC, D, L], F32, kind="ExternalOutput")
        with tile.TileContext(nc) as tc:
            with tc.tile_pool(name="sb", bufs=1) as sb:
                t = sb.tile([128, 4], F32, name="t")
                nc.vector.memset(t[:, :], 0.0)
                nc.sync.dma_start(out=resT[0, 0:128, 0:4], in_=t[:, :])
        nc.compile()
        _NULL_CACHE["r"] = _JitRunner(nc)
    return _NULL_CACHE["r"]


def time_null(inputs, reps=6):
    runner = _get_null_runner()
    in_maps = make_in_maps(**inputs)
    return runner.time_resident(in_maps, reps=reps)


def time_main(inputs, reps=6, dbg=False):
    runner = _get_runner(dbg=dbg)
    in_maps = make_in_maps(**inputs)
    return runner.time_resident(in_maps, reps=reps)


KT = 9  # repeat count for the slope-timing build


def time_exec(inputs, reps=10, dbg=False):
    """Per-execution NEFF time via the repeat-slope method.

    Dispatch latency through the axon relay is ~80ms with ±20ms jitter —
    far above the kernel's actual hardware time — so single-call timing
    (even null-subtracted) cannot resolve it. Instead build the identical
    kernel with the whole computation repeated KT times inside one NEFF;
    (t_KT - t_1) / (KT - 1) is exactly one full hardware execution
    (including all HBM loads/stores and the AllReduce, which sit inside
    the repeat loop), with dispatch cost cancelled.
    """
    in_maps = make_in_maps(**inputs)
    r1 = _get_runner(dbg=dbg)
    rk = _get_runner(dbg=dbg, kreps=KT)
    t1 = r1.time_resident(in_maps, reps=reps)
    tk = rk.time_resident(in_maps, reps=reps)
    per_exec = (tk - t1) / (KT - 1)
    return per_exec, t1, tk



# revision 15
# speedup vs baseline: 480.6797x; 1.1239x over previous
"""Autoformer encoder layer on 8 Trainium2 NeuronCores (Bass/Tile).

Data-parallel over batch: each of the 8 cores processes 4 of the 32 batches.
Everything runs on-device in one NEFF, including the cross-core reduction for
the global top-k delay selection (AllReduce of the batch-summed correlation
spectrum) and the data-dependent delay rolls (dynamic-offset SBUF reads).

Math notes
----------
The reference only ever uses two reductions of the full per-(head,channel)
autocorrelation:
  * mean_value[b, l] = mean over channels of irfft(rfft(q)*conj(rfft(k)))
    -> computed here as S[b,f] = sum_c Qf*conj(Kf) via DFT-as-matmul over L,
       then one tiny irfft matmul.  (The full [B,H,E,L] corr is never needed.)
  * agg = sum_i softmax(w)_i * roll(v, -d_i)  -> rolls commute with the
    output projection, so v@Wo is computed directly with folded weights
    Wvo = Wv @ Wo and rolled instead (7 dynamic-slice MACs per channel chunk).
The moving-average decomposition runs as a cumsum scan along the free axis in
channel-major layout, so the whole residual/FFN pipeline needs no on-device
transposes: x is fed pre-transposed as [B, D, L] and the output is returned
transposed, undone on the host.
"""

import os
from contextlib import ExitStack

import numpy as np

import concourse.bass as bass
import concourse.bacc as bacc
import concourse.mybir as mybir
from concourse import tile
from concourse.bass_utils import run_bass_kernel_spmd
from concourse.ordered_set import OrderedSet

F32 = mybir.dt.float32
F32R = mybir.dt.float32r
U32 = mybir.dt.uint32
AX = mybir.AxisListType
OP = mybir.AluOpType
AF = mybir.ActivationFunctionType
DVE = mybir.EngineType.DVE
SPROD_MODE = os.environ.get("KSPROD", "stt")

B, L, D, DFF = 32, 1536, 512, 2048
KMA = 25              # moving-average window
PAD = (KMA - 1) // 2  # 12
TOPK = 7              # int(1 * log(1536))
N_CORES = 8
BC = B // N_CORES     # batches per core
LF = L // 2 + 1       # 769 rfft bins
NLC = L // 128        # 12 l-chunks
NDC = D // 128        # 4 channel chunks
NFC = DFF // 128      # 16 ffn chunks
NFT = (LF + 127) // 128  # 7 f-tiles (last has 1 row)
NLB = L // 512        # 3 l-blocks of 512


def _host_consts():
    lv = np.arange(L)
    fv = np.arange(LF)
    ang = 2.0 * np.pi * np.outer(lv, fv) / L
    Cc = np.cos(ang).astype(np.float32)           # [L, LF] rfft real basis
    Cs = (-np.sin(ang)).astype(np.float32)        # [L, LF] rfft imag basis
    # irfft with the channel-mean folded in:
    # mv[l] = (1/(L*D)) sum_f beta_f (Sre[f] cos(wfl) - Sim[f] sin(wfl))
    # moving-average edge coefficients, pre-negated for fused (coef*edge)+rest
    coefL = np.tile((-(PAD - np.arange(PAD)) / KMA).astype(np.float32), (128, 1))
    coefR = np.tile((-(np.arange(PAD) + 1) / KMA).astype(np.float32), (128, 1))
    return Cc, Cs, None, coefL, coefR


def build(dbg=False, kreps=None):
    phases = int(os.environ.get("KPHASES", "2"))
    p1ft = int(os.environ.get("KP1FT", str(NFT)))
    p1b = int(os.environ.get("KP1B", str(BC)))
    if kreps is None:
        kreps = int(os.environ.get("KREPS", "1"))
    kar = int(os.environ.get("KAR", "1"))
    nc = bacc.Bacc("TRN2", target_bir_lowering=False, debug=False, num_devices=N_CORES)

    xT = nc.dram_tensor("xT", [BC, D, L], F32R, kind="ExternalInput")
    Wq_d = nc.dram_tensor("Wq", [D, D], F32R, kind="ExternalInput")
    Wk_d = nc.dram_tensor("Wk", [D, D], F32R, kind="ExternalInput")
    Wvo_d = nc.dram_tensor("Wvo", [D, D], F32R, kind="ExternalInput")
    W1_d = nc.dram_tensor("W1", [D, DFF], F32R, kind="ExternalInput")
    W2_d = nc.dram_tensor("W2", [DFF, D], F32R, kind="ExternalInput")
    bq_d = nc.dram_tensor("bq", [1, D], F32, kind="ExternalInput")
    bk_d = nc.dram_tensor("bk", [1, D], F32, kind="ExternalInput")
    # channel-major biases prepacked host-side as [128, nchunks]
    bvo_d = nc.dram_tensor("bvo", [128, NDC], F32, kind="ExternalInput")
    b1_d = nc.dram_tensor("b1", [128, NFC], F32, kind="ExternalInput")
    b2_d = nc.dram_tensor("b2", [128, NDC], F32, kind="ExternalInput")
    # shape-derived constants: baked into the NEFF, no per-call transfer
    Cc_np, Cs_np, _, coefL_np, coefR_np = _host_consts()
    Cc_d = nc.inline_tensor(Cc_np, name="Cc")
    Cs_d = nc.inline_tensor(Cs_np, name="Cs")
    coefL_d = nc.inline_tensor(coefL_np, name="coefL")
    coefR_d = nc.inline_tensor(coefR_np, name="coefR")

    resT = nc.dram_tensor("resT", [BC, D, L], F32, kind="ExternalOutput")
    if dbg:
        s_dbg = nc.dram_tensor("s_dbg", [128, 10 * NFT], F32, kind="ExternalOutput")
        mv_dbg = nc.dram_tensor("mv_dbg", [5, L], F32, kind="ExternalOutput")
        idx_dbg = nc.dram_tensor("idx_dbg", [1, 8], U32, kind="ExternalOutput")
        w_dbg = nc.dram_tensor("w_dbg", [BC, TOPK], F32, kind="ExternalOutput")

    with tile.TileContext(nc) as tc, ExitStack() as stack:
        pp = stack.enter_context(tc.tile_pool(name="persist", bufs=1))
        dram = stack.enter_context(tc.tile_pool(name="dram", bufs=1, space="DRAM"))

        # ---- persistent biases --------------------------------------------
        bq_bc = pp.tile([128, D], F32, tag="bqbc")
        bk_bc = pp.tile([128, D], F32, tag="bkbc")
        brow = pp.tile([1, D], F32, tag="brow")
        nc.sync.dma_start(out=brow[:, :], in_=bq_d[:, :])
        nc.gpsimd.partition_broadcast(bq_bc[:, :], brow[0:1, :])
        brow2 = pp.tile([1, D], F32, tag="brow2")
        nc.sync.dma_start(out=brow2[:, :], in_=bk_d[:, :])
        nc.gpsimd.partition_broadcast(bk_bc[:, :], brow2[0:1, :])

        bvoT = pp.tile([128, NDC], F32, tag="bvoT")
        b1T = pp.tile([128, NFC], F32, tag="b1T")
        b2T = pp.tile([128, NDC], F32, tag="b2T")
        nc.sync.dma_start(out=bvoT[:, :], in_=bvo_d[:, :])
        nc.sync.dma_start(out=b1T[:, :], in_=b1_d[:, :])
        nc.sync.dma_start(out=b2T[:, :], in_=b2_d[:, :])

        coefL_sb = pp.tile([128, PAD], F32, tag="coefL")
        coefR_sb = pp.tile([128, PAD], F32, tag="coefR")
        nc.sync.dma_start(out=coefL_sb[:, :], in_=coefL_d[:, :])
        nc.sync.dma_start(out=coefR_sb[:, :], in_=coefR_d[:, :])

        # identity matrix for PE transposes + irfft row scales
        idint = pp.tile([128, 128], mybir.dt.int32, tag="idint")
        nc.gpsimd.iota(idint[:, :], pattern=[[1, 128]], base=0, channel_multiplier=-1)
        ident = pp.tile([128, 128], F32, tag="ident")
        nc.vector.tensor_scalar(ident[:, :], idint[:, :], 0, None, op0=OP.is_equal)
        betac = pp.tile([128, 1], F32, tag="betac")
        nc.vector.memset(betac[:, :], 2.0 / (L * D))
        betac0 = pp.tile([128, 1], F32, tag="betac0")
        nc.vector.memset(betac0[:, :], 2.0 / (L * D))
        nc.vector.memset(betac0[0:1, :], 1.0 / (L * D))

        # spectrum accumulator: 7 f-tiles x (4 batches + batchsum) columns
        s_re = pp.tile([128, 5 * NFT], F32, tag="s_re")
        s_im = pp.tile([128, 5 * NFT], F32, tag="s_im")
        nc.vector.memset(s_re[:, :], 0.0)
        nc.vector.memset(s_im[:, :], 0.0)

        mv_sb = pp.tile([5, L], F32, tag="mv")  # rows 0-3: batches, 4: batchsum
        idx_sb = pp.tile([1, 8], U32, tag="idx")
        wbc = pp.tile([128, BC * TOPK], F32, tag="wbc")
        wv = pp.tile([BC, TOPK], F32, tag="wv")

        for _rep in range(kreps):
            # ================= PHASE 1: q/k projections + DFT spectrum =========
            with ExitStack() as p1stack:
                p1c = p1stack.enter_context(tc.tile_pool(name="p1c", bufs=1))
                wq_sb, wk_sb = [], []
                for c in range(NDC):
                    t = p1c.tile([128, D], F32R, tag=f"wq{c}")
                    nc.sync.dma_start(out=t[:, :], in_=Wq_d[128 * c : 128 * (c + 1), :])
                    wq_sb.append(t)
                    t = p1c.tile([128, D], F32R, tag=f"wk{c}")
                    nc.sync.dma_start(out=t[:, :], in_=Wk_d[128 * c : 128 * (c + 1), :])
                    wk_sb.append(t)
                cc_sb, cs_sb = [], []
                for lc in range(NLC):
                    t = p1c.tile([128, LF], F32R, tag=f"cc{lc}")
                    nc.sync.dma_start(
                        out=t[:, :],
                        in_=Cc_d[128 * lc : 128 * (lc + 1), :].bitcast(F32R))
                    cc_sb.append(t)
                    t = p1c.tile([128, LF], F32R, tag=f"cs{lc}")
                    nc.sync.dma_start(
                        out=t[:, :],
                        in_=Cs_d[128 * lc : 128 * (lc + 1), :].bitcast(F32R))
                    cs_sb.append(t)

                pbatch = ExitStack()
                p1x = pbatch.enter_context(tc.tile_pool(name="p1x", bufs=1))
                p1q = pbatch.enter_context(tc.tile_pool(name="p1q", bufs=1))
                p1e = pbatch.enter_context(tc.tile_pool(name="p1e", bufs=1))
                p1s = pbatch.enter_context(tc.tile_pool(name="p1s", bufs=2))
                ps1 = pbatch.enter_context(tc.tile_pool(name="psum1", bufs=2, space="PSUM"))
                ps1d = pbatch.enter_context(tc.tile_pool(name="psum1d", bufs=1, space="PSUM"))

                for b in range(p1b):
                    xt = [p1x.tile([128, L], F32R, tag=f"xt{c}", name=f"xt{c}") for c in range(NDC)]
                    for c in range(NDC):
                        nc.sync.dma_start(
                            out=xt[c][:, :], in_=xT[b, 128 * c : 128 * (c + 1), :])

                    q_sb, k_sb = [], []
                    for lt in range(NLC):
                        pq = ps1.tile([128, D], F32, tag="pq")
                        pk = ps1.tile([128, D], F32, tag="pk")
                        for c in range(NDC):
                            nc.tensor.matmul(
                                pq[:, :], xt[c][:, 128 * lt : 128 * (lt + 1)],
                                wq_sb[c][:, :], start=(c == 0), stop=(c == NDC - 1))
                        for c in range(NDC):
                            nc.tensor.matmul(
                                pk[:, :], xt[c][:, 128 * lt : 128 * (lt + 1)],
                                wk_sb[c][:, :], start=(c == 0), stop=(c == NDC - 1))
                        qt = p1q.tile([128, D], F32R, tag=f"q{lt}")
                        kt = p1q.tile([128, D], F32R, tag=f"k{lt}")
                        nc.vector.tensor_add(qt[:, :], pq[:, :], bq_bc[:, :])
                        nc.vector.tensor_add(kt[:, :], pk[:, :], bk_bc[:, :])
                        q_sb.append(qt)
                        k_sb.append(kt)

                    for ft in range(p1ft):
                        m = min(128, LF - 128 * ft)
                        fsl = slice(128 * ft, 128 * ft + m)
                        pqr = ps1d.tile([128, D], F32, tag="pqr")
                        pqi = ps1d.tile([128, D], F32, tag="pqi")
                        pkr = ps1d.tile([128, D], F32, tag="pkr")
                        pki = ps1d.tile([128, D], F32, tag="pki")
                        for lc in range(NLC):
                            st_ = lc == 0
                            sp_ = lc == NLC - 1
                            nc.tensor.matmul(pqr[:m, :], cc_sb[lc][:, fsl],
                                             q_sb[lc][:, :], start=st_, stop=sp_)
                            nc.tensor.matmul(pqi[:m, :], cs_sb[lc][:, fsl],
                                             q_sb[lc][:, :], start=st_, stop=sp_)
                            nc.tensor.matmul(pkr[:m, :], cc_sb[lc][:, fsl],
                                             k_sb[lc][:, :], start=st_, stop=sp_)
                            nc.tensor.matmul(pki[:m, :], cs_sb[lc][:, fsl],
                                             k_sb[lc][:, :], start=st_, stop=sp_)
                        # DVE reads at most one PSUM operand per instruction:
                        # stage the k-side spectra in SBUF, q-side stays PSUM
                        kr = p1e.tile([128, D], F32, tag="kr")
                        ki = p1e.tile([128, D], F32, tag="ki")
                        nc.scalar.copy(kr[:m, :], pkr[:m, :])
                        nc.scalar.copy(ki[:m, :], pki[:m, :])
                        scr = p1s.tile([128, D], F32, tag="scr")
                        acc = p1s.tile([128, 4], F32, tag="acc")
                        col = 5 * ft + b
                        prods = ((pqr, kr, 0), (pqi, ki, 1),
                                 (pqi, kr, 2), (pqr, ki, 3))
                        if SPROD_MODE == "stt":
                            for pa, pb, j in prods:
                                nc.vector.scalar_tensor_tensor(
                                    out=scr[:m, :], in0=pa[:m, :], scalar=1.0,
                                    in1=pb[:m, :], op0=OP.mult, op1=OP.mult,
                                    accum_out=acc[:m, j : j + 1])
                        else:
                            for pa, pb, j in prods:
                                nc.vector.tensor_mul(scr[:m, :], pa[:m, :], pb[:m, :])
                                nc.vector.reduce_sum(
                                    acc[:m, j : j + 1], scr[:m, :], axis=AX.X)
                        nc.vector.tensor_add(
                            s_re[:m, col : col + 1], acc[:m, 0:1], acc[:m, 1:2])
                        nc.vector.tensor_sub(
                            s_im[:m, col : col + 1], acc[:m, 2:3], acc[:m, 3:4])

                pbatch.close()
                # ================= PHASE 1.5: irfft, allreduce, topk, weights ======
                if dbg:
                    nc.sync.dma_start(out=s_dbg[:, 0 : 5 * NFT], in_=s_re[:, :])
                    nc.sync.dma_start(out=s_dbg[:, 5 * NFT : 10 * NFT], in_=s_im[:, :])
                if phases < 0:
                    nc.vector.memset(mv_sb[:, :], 0.0)
                    nc.vector.memset(idx_sb[:, :], 0)
                    nc.vector.memset(wbc[:, :], 0.0)
                    nc.vector.memset(wv[:, :], 0.0)
                nftr = range(NFT) if phases >= 0 else range(0)
                for ft in nftr:
                    nc.vector.reduce_sum(
                        s_re[:, 5 * ft + 4 : 5 * ft + 5], s_re[:, 5 * ft : 5 * ft + 4],
                        axis=AX.X)
                    nc.vector.reduce_sum(
                        s_im[:, 5 * ft + 4 : 5 * ft + 5], s_im[:, 5 * ft : 5 * ft + 4],
                        axis=AX.X)

                with (
                    tc.tile_pool(name="irf", bufs=3) as irf,
                    tc.tile_pool(name="psum15", bufs=2, space="PSUM") as ps15,
                    tc.tile_pool(name="psum15t", bufs=2, space="PSUM") as ps15t,
                ):
                    # Cc/Cs are still resident; derive the irfft operand
                    # (beta_f/(L*D) * [cos|-sin]^T) by transposing their tiles.
                    for nb in range(NLB) if phases >= 0 else range(0):
                        pmv = ps15.tile([5, 512], F32, tag="pmv")
                        first = True
                        for half, stile, csb in ((0, s_re, cc_sb), (1, s_im, cs_sb)):
                            for ft in range(NFT):
                                m = min(128, LF - 128 * ft)
                                fsl = slice(128 * ft, 128 * ft + m)
                                pt = ps15t.tile([128, 512], F32, tag="pt")
                                for j in range(4):
                                    lc = 4 * nb + j
                                    nc.tensor.transpose(
                                        pt[:m, 128 * j : 128 * (j + 1)],
                                        csb[lc][:, fsl].bitcast(F32),
                                        ident[:, :])
                                mt = irf.tile([128, 512], F32, tag="minv")
                                if m < 128:
                                    nc.vector.memset(mt[:, :], 0.0)
                                bcol = betac0 if (ft == 0 or ft == NFT - 1) else betac
                                nc.scalar.activation(
                                    mt[:m, :], pt[:m, :], AF.Copy, scale=bcol[:m, 0:1])
                                nc.tensor.matmul(
                                    pmv[:, :], stile[:, 5 * ft : 5 * ft + 5], mt[:, :],
                                    start=first, stop=(half == 1 and ft == NFT - 1))
                                first = False
                        nc.vector.tensor_copy(mv_sb[:, 512 * nb : 512 * (nb + 1)], pmv[:, :])

            # AllReduce the batch-summed spectrum row -> global over all 32 batches
            do_ar = phases >= 0 and kar != 0
            cc_in = dram.tile([1, L], F32)
            cc_out = dram.tile([1, L], F32)
            mvg = pp.tile([1, L], F32, tag="mvg")
            max8 = pp.tile([1, 8], F32, tag="max8")
            if phases >= 0 and not do_ar:
                nc.vector.memset(idx_sb[:, :], 0)
            if do_ar:
                nc.sync.dma_start(out=cc_in[:, :], in_=mv_sb[4:5, :])
                nc.gpsimd.collective_compute(
                    "AllReduce",
                    OP.add,
                    replica_groups=[list(range(N_CORES))],
                    ins=[cc_in[:, :].opt()],
                    outs=[cc_out[:, :].opt()],
                )
                nc.sync.dma_start(out=mvg[:, :], in_=cc_out[:, :])
                nc.vector.max(out=max8[:, :], in_=mvg[:, :])
                nc.vector.max_index(out=idx_sb[:, :], in_max=max8[:, :], in_values=mvg[:, :])

            ntk = range(TOPK) if phases >= 1 else range(0)
            dvals = [
                nc.values_load(
                    idx_sb[0:1, i : i + 1],
                    engines=OrderedSet([DVE]),
                    min_val=0,
                    max_val=L - 1,
                    skip_runtime_bounds_check=True,
                )
                for i in ntk
            ]

            # per-batch weights at the selected delays + softmax, then broadcast
            if 0 <= phases < 1:
                nc.vector.memset(wv[:, :], 0.0)
            for i in ntk:
                nc.vector.tensor_copy(wv[:, i : i + 1], mv_sb[0:BC, bass.ds(dvals[i], 1)])
            wred = pp.tile([BC, 2], F32, tag="wred")
            if phases < 1:
                nc.vector.memset(wbc[:, :], 0.0)
            if phases >= 1:
                nc.vector.reduce_max(wred[:, 0:1], wv[:, :], axis=AX.X)
                wexp = pp.tile([BC, TOPK], F32, tag="wexp")
                nc.vector.tensor_scalar(
                    wexp[:, :], wv[:, :], wred[:, 0:1], None, op0=OP.subtract)
                nc.scalar.activation(wexp[:, :], wexp[:, :], AF.Exp)
                nc.vector.reduce_sum(wred[:, 1:2], wexp[:, :], axis=AX.X)
                nc.vector.reciprocal(wred[:, 1:2], wred[:, 1:2])
                nc.vector.tensor_scalar(
                    wexp[:, :], wexp[:, :], wred[:, 1:2], None, op0=OP.mult)
                w_dram = dram.tile([BC, TOPK], F32)
                nc.sync.dma_start(out=w_dram[:, :], in_=wexp[:, :])
                wflat = pp.tile([1, BC * TOPK], F32, tag="wflat")
                nc.sync.dma_start(out=wflat[:, :], in_=w_dram[:, :])
                nc.gpsimd.partition_broadcast(wbc[:, :], wflat[0:1, :])

            if dbg:
                nc.sync.dma_start(out=mv_dbg[:, :], in_=mv_sb[:, :])
                nc.sync.dma_start(out=idx_dbg[:, :], in_=idx_sb[:, :])
                nc.sync.dma_start(out=w_dbg[:, :], in_=wexp[:, :] if phases >= 1 else wv[:, :])

            # ================= PHASE 2: rolls, decomp, FFN, decomp =============
            def ma_seasonal(pool, dst, src):
                """dst = src - moving_avg(src) along the free axis (edge-replicated).

                src must be F32-readable; dst may be F32 or F32R."""
                cs1 = pool.tile([128, L + 1], F32, tag="cs1", bufs=1)
                nc.vector.memset(cs1[:, 0:1], 0.0)
                nc.vector.tensor_tensor_scan(
                    cs1[:, 1 : L + 1], src[:, :], src[:, :], 0.0,
                    op0=OP.add, op1=OP.bypass)
                # left edge: s[l] = x[l] - cs1[l+PAD+1]/K - (PAD-l)/K * x[0]
                # (before the in-place window diff clobbers cs1[PAD+1:2*PAD+1])
                nc.vector.scalar_tensor_tensor(
                    out=dst[:, 0:PAD], in0=cs1[:, PAD + 1 : 2 * PAD + 1],
                    scalar=-1.0 / KMA, in1=src[:, 0:PAD], op0=OP.mult, op1=OP.add)
                nc.vector.scalar_tensor_tensor(
                    out=dst[:, 0:PAD], in0=coefL_sb[:, :], scalar=src[:, 0:1],
                    in1=dst[:, 0:PAD], op0=OP.mult, op1=OP.add)
                # right edge: s[l] = x[l] - (stot - cs1[l-PAD])/K - (l-L+PAD+1)/K * x[L-1]
                e2 = pool.tile([128, PAD], F32, tag="e2", bufs=1)
                nc.vector.tensor_scalar(
                    e2[:, :], cs1[:, L - 2 * PAD : L - PAD], cs1[:, L : L + 1],
                    1.0 / KMA, op0=OP.subtract, op1=OP.mult)
                nc.vector.tensor_add(
                    dst[:, L - PAD : L], e2[:, :], src[:, L - PAD : L])
                nc.vector.scalar_tensor_tensor(
                    out=dst[:, L - PAD : L], in0=coefR_sb[:, :],
                    scalar=src[:, L - 1 : L], in1=dst[:, L - PAD : L],
                    op0=OP.mult, op1=OP.add)
                # window diff written in place over cs1's low region: out[j]
                # reads cs1[j] and cs1[j+2*PAD+1]; elements process in
                # increasing order so the read-ahead slot is still unwritten
                dif = cs1[:, 0 : L - 2 * PAD]
                nc.vector.tensor_sub(
                    dif[:, :], cs1[:, 2 * PAD + 1 : L + 1], cs1[:, 0 : L - 2 * PAD])
                nc.vector.scalar_tensor_tensor(
                    out=dst[:, PAD : L - PAD], in0=dif[:, :], scalar=-1.0 / KMA,
                    in1=src[:, PAD : L - PAD], op0=OP.mult, op1=OP.add)

            with ExitStack() as p2stack:
                nch = range(NDC) if phases >= 2 else range(0)
                nfh = range(NFC) if phases >= 2 else range(0)
                p2w = p2stack.enter_context(tc.tile_pool(name="p2w", bufs=1))
                w1_sb = []
                for c in nch:
                    t = p2w.tile([128, DFF], F32R, tag=f"w1_{c}")
                    nc.sync.dma_start(out=t[:, :], in_=W1_d[128 * c : 128 * (c + 1), :])
                    w1_sb.append(t)
                w2_sb = []
                for c in nfh:
                    t = p2w.tile([128, D], F32R, tag=f"w2_{c}")
                    nc.sync.dma_start(out=t[:, :], in_=W2_d[128 * c : 128 * (c + 1), :])
                    w2_sb.append(t)
                wvo_sb = []
                for c in nch:
                    t = p2w.tile([128, D], F32R, tag=f"wvo{c}")
                    nc.sync.dma_start(out=t[:, :], in_=Wvo_d[128 * c : 128 * (c + 1), :])
                    wvo_sb.append(t)

                p2x = p2stack.enter_context(tc.tile_pool(name="p2x", bufs=1))
                p2 = p2stack.enter_context(tc.tile_pool(name="p2", bufs=1))
                ps2 = p2stack.enter_context(tc.tile_pool(name="psum2", bufs=2, space="PSUM"))
                ps2y = p2stack.enter_context(tc.tile_pool(name="psum2y", bufs=1, space="PSUM"))

                for b in range(BC) if phases >= 2 else range(0):
                    xt = [p2x.tile([128, L], F32R, tag=f"x2t{c}", name=f"x2t{c}") for c in range(NDC)]
                    for c in range(NDC):
                        nc.sync.dma_start(
                            out=xt[c][:, :], in_=xT[b, 128 * c : 128 * (c + 1), :])

                    # vo' = x @ (Wv Wo); x2 = x + bvo + sum_i w_i roll(vo', d_i)
                    x2 = [p2.tile([128, L], F32, tag=f"x2_{c}", name=f"x2_{c}") for c in range(NDC)]
                    for c in range(NDC):
                        vo2 = p2.tile([128, 2 * L], F32, tag="vo2", bufs=2)
                        for nb in range(NLB):
                            pv = ps2.tile([128, 512], F32, tag="pv")
                            for cx in range(NDC):
                                nc.tensor.matmul(
                                    pv[:, :],
                                    wvo_sb[cx][:, 128 * c : 128 * (c + 1)],
                                    xt[cx][:, 512 * nb : 512 * (nb + 1)],
                                    start=(cx == 0),
                                    stop=(cx == NDC - 1),
                                )
                            nc.scalar.copy(vo2[:, 512 * nb : 512 * (nb + 1)], pv[:, :])
                            nc.scalar.copy(
                                vo2[:, L + 512 * nb : L + 512 * (nb + 1)], pv[:, :])
                        nc.scalar.activation(
                            x2[c][:, :], xt[c][:, :], AF.Identity,
                            bias=bvoT[:, c : c + 1])
                        for i in range(TOPK):
                            nc.vector.scalar_tensor_tensor(
                                out=x2[c][:, :],
                                in0=vo2[:, bass.ds(dvals[i], L)],
                                scalar=wbc[:, TOPK * b + i : TOPK * b + i + 1],
                                in1=x2[c][:, :],
                                op0=OP.mult,
                                op1=OP.add,
                            )

                    # first decomposition -> seasonal part s (f32r: feeds FFN)
                    st = [p2.tile([128, L], F32R, tag=f"st{c}", name=f"st{c}") for c in range(NDC)]
                    for c in range(NDC):
                        ma_seasonal(p2, st[c], x2[c])

                    # FFN + residual: z = s + relu(s W1 + b1) W2 + b2
                    z = x2  # reuse buffers
                    for nb in range(NLB):
                        lsl = slice(512 * nb, 512 * (nb + 1))
                        py = [ps2y.tile([128, 512], F32, tag=f"py{c}", name=f"py{c}") for c in range(NDC)]
                        for fc in range(NFC):
                            ph = ps2.tile([128, 512], F32, tag="ph")
                            for c in range(NDC):
                                nc.tensor.matmul(
                                    ph[:, :],
                                    w1_sb[c][:, 128 * fc : 128 * (fc + 1)],
                                    st[c][:, lsl],
                                    start=(c == 0),
                                    stop=(c == NDC - 1),
                                )
                            ht = p2.tile([128, 512], F32R, tag="ht", bufs=3)
                            nc.scalar.activation(
                                ht[:, :], ph[:, :], AF.Relu, bias=b1T[:, fc : fc + 1])
                            for c in range(NDC):
                                nc.tensor.matmul(
                                    py[c][:, :],
                                    w2_sb[fc][:, 128 * c : 128 * (c + 1)],
                                    ht[:, :],
                                    start=(fc == 0),
                                    stop=(fc == NFC - 1),
                                )
                        for c in range(NDC):
                            nc.vector.scalar_tensor_tensor(
                                out=z[c][:, lsl], in0=py[c][:, :],
                                scalar=b2T[:, c : c + 1], in1=st[c][:, lsl],
                                op0=OP.add, op1=OP.add)

                    # second decomposition -> output (reuses st[c], dead after FFN)
                    for c in range(NDC):
                        rt = st[c]
                        ma_seasonal(p2, rt, z[c])
                        nc.sync.dma_start(
                            out=resT[b, 128 * c : 128 * (c + 1), :],
                            in_=rt[:, :].bitcast(F32))

    nc.compile()
    return nc


_CACHE = {}


def _get_nc(dbg=False, kreps=None):
    key = (dbg, kreps)
    if key not in _CACHE:
        _CACHE[key] = build(dbg=dbg, kreps=kreps)
    return _CACHE[key]


def make_in_maps(x, Wq, bq, Wk, bk, Wv, bv, Wo, bo, W1, b1, W2, b2):
    x = np.asarray(x, np.float32)
    Wvo = (np.asarray(Wv, np.float64) @ np.asarray(Wo, np.float64)).astype(np.float32)
    bvo = (np.asarray(bv, np.float64) @ np.asarray(Wo, np.float64)
           + np.asarray(bo, np.float64)).astype(np.float32)
    shared = {
        "Wq": np.ascontiguousarray(Wq, np.float32),
        "Wk": np.ascontiguousarray(Wk, np.float32),
        "Wvo": Wvo,
        "W1": np.ascontiguousarray(W1, np.float32),
        "W2": np.ascontiguousarray(W2, np.float32),
        "bq": np.asarray(bq, np.float32).reshape(1, D),
        "bk": np.asarray(bk, np.float32).reshape(1, D),
        "bvo": np.ascontiguousarray(bvo.reshape(NDC, 128).T),
        "b1": np.ascontiguousarray(np.asarray(b1, np.float32).reshape(NFC, 128).T),
        "b2": np.ascontiguousarray(np.asarray(b2, np.float32).reshape(NDC, 128).T),
    }
    in_maps = []
    for c in range(N_CORES):
        xs = x[BC * c : BC * (c + 1)]
        in_maps.append({**shared, "xT": np.ascontiguousarray(xs.transpose(0, 2, 1))})
    return in_maps


class _JitRunner:
    """Holds a jitted shard_map for a compiled Bass module, so repeat calls
    skip retrace/relower/XLA-recompile (run_bass_kernel_spmd rebuilds the jit
    — and re-serializes the whole BIR into the custom call — on every call).

    Zero-init output buffers are created ON DEVICE (tiny jitted memset) and
    donated, instead of shipping host zeros every call.
    """

    def __init__(self, nc):
        import jax
        import jax.numpy as jnp
        from jax.sharding import Mesh, PartitionSpec, NamedSharding
        from jax.experimental.shard_map import shard_map
        from concourse import bass2jax

        bass2jax.install_neuronx_cc_hook()
        self.jax = jax
        self.nc = nc
        partition_name = (
            nc.partition_id_tensor.name if nc.partition_id_tensor else None)
        in_names, out_names, out_avals, out_shapes = [], [], [], []
        for alloc in nc.m.functions[0].allocations:
            if not isinstance(alloc, mybir.MemoryLocationSet):
                continue
            if alloc.kind == "ExternalInput":
                name = alloc.memorylocations[0].name
                if name != partition_name:
                    in_names.append(name)
            elif alloc.kind == "ExternalOutput":
                shape = tuple(alloc.tensor_shape)
                dtype = mybir.dt.np(alloc.dtype)
                out_avals.append(jax.core.ShapedArray(shape, dtype))
                out_names.append(alloc.memorylocations[0].name)
                out_shapes.append((shape, dtype))
        self.in_names = in_names
        self.out_names = out_names
        self.out_avals = out_avals
        n_params = len(in_names)
        n_outs = len(out_names)
        in_names_all = in_names + out_names
        if partition_name is not None:
            in_names_all.append(partition_name)

        def _body(*args):
            operands = list(args)
            if partition_name is not None:
                operands.append(bass2jax.partition_id_tensor())
            outs = bass2jax._bass_exec_p.bind(
                *operands,
                out_avals=tuple(out_avals),
                in_names=tuple(in_names_all),
                out_names=tuple(out_names),
                lowering_input_output_aliases=(),
                sim_require_finite=True,
                sim_require_nnan=True,
                nc=nc,
            )
            return tuple(outs)

        devices = jax.devices()[:N_CORES]
        mesh = Mesh(np.asarray(devices), ("core",))
        self.sharding = NamedSharding(mesh, PartitionSpec("core"))
        in_specs = (PartitionSpec("core"),) * (n_params + n_outs)
        out_specs = (PartitionSpec("core"),) * n_outs
        self.sharded = jax.jit(
            shard_map(_body, mesh=mesh, in_specs=in_specs,
                      out_specs=out_specs, check_rep=False),
            donate_argnums=tuple(range(n_params, n_params + n_outs)),
            keep_unused=True)

        def _mk_zeros():
            return tuple(
                jnp.zeros((N_CORES * s[0], *s[1:]), dt) for s, dt in out_shapes)

        self._zeros = jax.jit(
            _mk_zeros, out_shardings=(self.sharding,) * n_outs)

    def concat_inputs(self, in_maps):
        per_core = [[np.asarray(m[name]) for name in self.in_names]
                    for m in in_maps]
        return [np.concatenate([per_core[c][i] for c in range(N_CORES)], axis=0)
                for i in range(len(self.in_names))]

    def zeros(self):
        z = self._zeros()
        self.jax.block_until_ready(z)
        return z

    def call(self, args):
        outs = self.sharded(*args, *self.zeros())
        self.jax.block_until_ready(outs)
        return outs

    def run_np(self, in_maps):
        outs = self.call(self.concat_inputs(in_maps))
        return [
            {name: np.asarray(outs[i]).reshape(
                N_CORES, *self.out_avals[i].shape)[c]
             for i, name in enumerate(self.out_names)}
            for c in range(N_CORES)
        ]

    def time_resident(self, in_maps, reps=6):
        import time as _time
        jax = self.jax
        dev_in = [jax.device_put(a, self.sharding)
                  for a in self.concat_inputs(in_maps)]
        jax.block_until_ready(dev_in)
        self.call(dev_in)  # warmup (compile on first use)
        best = float("inf")
        for _ in range(reps):
            z = self.zeros()
            t0 = _time.time()
            outs = self.sharded(*dev_in, *z)
            jax.block_until_ready(outs)
            best = min(best, _time.time() - t0)
        return best


_RUNNERS = {}


def _get_runner(dbg=False, kreps=None):
    key = (dbg, kreps)
    if key not in _RUNNERS:
        _RUNNERS[key] = _JitRunner(_get_nc(dbg=dbg, kreps=kreps))
    return _RUNNERS[key]


class _Res:
    def __init__(self, results):
        self.results = results
        self.exec_time_ns = None
        self.instructions_and_trace = None
        self.profile_json = None


def run(inputs, dbg=False, trace=False):
    runner = _get_runner(dbg=dbg)
    in_maps = make_in_maps(**inputs)
    results = runner.run_np(in_maps)
    out = np.empty((B, L, D), np.float32)
    for c in range(N_CORES):
        rt = results[c]["resT"]  # [BC, D, L]
        out[BC * c : BC * (c + 1)] = rt.transpose(0, 2, 1)
    return out, _Res(results)


def kernel(**inputs):
    out, _ = run(inputs)
    return out


_NULL_CACHE = {}


def _get_null_runner():
    if "r" not in _NULL_CACHE:
        nc = bacc.Bacc("TRN2", target_bir_lowering=False, debug=False,
                       num_devices=N_CORES)
        ins = {
            "xT": [BC, D, L], "Wq": [D, D], "Wk": [D, D], "Wvo": [D, D],
            "W1": [D, DFF], "W2": [DFF, D], "bq": [1, D], "bk": [1, D],
            "bvo": [128, NDC], "b1": [128, NFC], "b2": [128, NDC],
        }
        for name, shape in ins.items():
            nc.dram_tensor(name, shape, F32, kind="ExternalInput")
        resT = nc.dram_tensor("resT", [BC, D, L], F32, kind="ExternalOutput")
        with tile.TileContext(nc) as tc:
            with tc.tile_pool(name="sb", bufs=1) as sb:
                t = sb.tile([128, 4], F32, name="t")
                nc.vector.memset(t[:, :], 0.0)
                nc.sync.dma_start(out=resT[0, 0:128, 0:4], in_=t[:, :])
        nc.compile()
        _NULL_CACHE["r"] = _JitRunner(nc)
    return _NULL_CACHE["r"]


def time_null(inputs, reps=6):
    runner = _get_null_runner()
    in_maps = make_in_maps(**inputs)
    return runner.time_resident(in_maps, reps=reps)


def time_main(inputs, reps=6, dbg=False):
    runner = _get_runner(dbg=dbg)
    in_maps = make_in_maps(**inputs)
    return runner.time_resident(in_maps, reps=reps)


KT = 9  # repeat count for the slope-timing build


def time_exec(inputs, reps=10, dbg=False):
    """Per-execution NEFF time via the repeat-slope method.

    Dispatch latency through the axon relay is ~80ms with ±20ms jitter —
    far above the kernel's actual hardware time — so single-call timing
    (even null-subtracted) cannot resolve it. Instead build the identical
    kernel with the whole computation repeated KT times inside one NEFF;
    (t_KT - t_1) / (KT - 1) is exactly one full hardware execution
    (including all HBM loads/stores and the AllReduce, which sit inside
    the repeat loop), with dispatch cost cancelled.
    """
    in_maps = make_in_maps(**inputs)
    r1 = _get_runner(dbg=dbg)
    rk = _get_runner(dbg=dbg, kreps=KT)
    t1 = r1.time_resident(in_maps, reps=reps)
    tk = rk.time_resident(in_maps, reps=reps)
    per_exec = (tk - t1) / (KT - 1)
    return per_exec, t1, tk

